# revision 36
# baseline (speedup 1.0000x reference)
"""Trainium2 Bass kernel for nn_GCFNN (2-modality GCN+GAT VAE-ish net).

v6 strategy (column-sharded adjacency + software-pipelined epilogue):
- Column-shard the adjacency: core c holds adjTc = adj[:, cS:(c+1)S].T
  ([512 own-j, 4096 all-i], j on partitions) resident in SBUF. Each GCN layer
  computes the support for the core's own 512 nodes locally, then its partial
  aggregation over all 4096 output rows, combined with a per-layer per-mod
  ReduceScatter(add) (0.5 MB out/core instead of a 4 MB AllGather).
- GAT attention stays column-sharded: logits need only a per-mod [N]-vector
  AllGather of a1h; exp/mask/att@h partials plus a ones-row denominator go
  through ONE merged (both-modality) attention ReduceScatter.
- Engine balance: PSUM eviction copies alternate DVE/ACT; the mask-multiply
  is a single 3-D strided DVE op per chunk; exp batches all 4 j-tiles into
  one wide ACT op. (Pool/GPSIMD is compute-useless here: it cannot touch
  PSUM, rejects TensorScalarPtr, and real-HW op launch costs ~6 us.)
- Software pipelining: rep r's epilogue (E/predictors/PoE, which wait on the
  attention RS) is emitted after rep r+1's compute so its long-latency waits
  do not stall the next rep's work in the in-order engine queues; rsA DRAM is
  double-buffered by rep parity. For the real K_REPS=1 build this is a no-op
  reordering. Collectives (Pool queue, the only legal engine; max 2 in
  flight) are ordered so the last trigger of a rep is preceded by cheap AGs,
  reopening the 2-deep trigger window early for the next rep.
- DMA batching: RS-input writes move 4 PSUM tiles per descriptor; rs_out and
  a1h reads are single descriptors.
- Steady-state pipelining: the next rep's GCN1 (supports + aggregation +
  RS1 triggers) is emitted interleaved into the current rep's attention
  chunks (PE fill + early RS1 fire); streaming rings for the RS-input
  staging tile and the attention mask product are 3 deep.
Measured (slope method, see test.py): ~212 us/forward steady-state on the
8-core ring (v3 baseline: ~277 us; wall-clock prints are dominated by a
~8.6 ms axon RPC dispatch floor and do not reflect device time).
"""

import functools
import os
import sys

import numpy as np

if "/opt/trn_rl_repo" not in sys.path:
    sys.path.insert(0, "/opt/trn_rl_repo")

import concourse.bacc as bacc
import concourse.mybir as mybir
import concourse.tile as tile
from concourse.bass_interp import get_hw_module
from concourse.bass_utils import run_bass_kernel_spmd

N, D, H, F2, Z, Y, PH, M = 4096, 400, 256, 128, 64, 5, 128, 2
NCORES = 8
S = N // NCORES          # 512 rows per core
NJT = S // 128           # 4 own j-tiles
NIB = S // 128           # 4 own i-tiles
DP = 512                 # D padded to 4*128
NDT = DP // 128          # 4 d-tiles
NFB = H // 128           # 2 feature blocks
NCH = N // S             # 8 aggregation i-chunks of 512
ARS = F2 + 2             # att RS block rows: 128 out + 1 den + 1 pad
EPS = 1e-8

F32 = mybir.dt.float32
F32R = mybir.dt.float32r
BF16 = mybir.dt.bfloat16

MM_BF16 = os.environ.get("K_MM_BF16", "1") == "1"   # bf16 adj/x/W matmuls
MM_DT = BF16 if MM_BF16 else F32R
RS_DT = BF16
AFT = mybir.ActivationFunctionType
ALU = mybir.AluOpType

NO_COLL = os.environ.get("K_NO_COLL", "0") == "1"


def _emit(nc, tc, P):
    """Emit the whole per-core program. P = dict of DRAM param APs."""
    rs1_in, rs1_out, rs2_in, rs2_out = [], [], [], []
    a1h_in, a1h_out, rsA_in, rsA_out = [], [], [], []
    a2h_d = []
    for m in range(M):
        rs1_in.append(nc.dram_tensor(f"rs1_in{m}", [NCH * H, S], RS_DT))
        rs1_out.append(nc.dram_tensor(f"rs1_out{m}", [H, S], RS_DT))
        rs2_in.append(nc.dram_tensor(f"rs2_in{m}", [NCH * H, S], RS_DT))
        rs2_out.append(nc.dram_tensor(f"rs2_out{m}", [H, S], RS_DT))
        a1h_in.append(nc.dram_tensor(f"a1h_in{m}", [S, 1], F32R))
        a1h_out.append(nc.dram_tensor(f"a1h_out{m}", [N, 1], F32R,
                                      addr_space="Shared"))
        a2h_d.append(nc.dram_tensor(f"a2h_d{m}", [S, 1], F32))
    # double-buffered (by rep parity) so the software-pipelined epilogue of
    # rep r-1 can read while rep r's collective writes the other buffer
    rsA_in = [nc.dram_tensor(f"rsA_in{p}", [NCH * M * ARS, S], RS_DT)
              for p in range(2)]
    rsA_out = [nc.dram_tensor(f"rsA_out{p}", [M * ARS, S], RS_DT)
               for p in range(2)]

    rg = [list(range(NCORES))]

    with (
        tc.tile_pool(name="persist", bufs=1) as pp,
        tc.tile_pool(name="stream", bufs=4) as sp,
        tc.tile_pool(name="work", bufs=3) as wp,
        tc.tile_pool(name="pmpool", bufs=3) as pmp,
        tc.tile_pool(name="small", bufs=4) as smp,
        tc.tile_pool(name="ps512", bufs=4, space="PSUM") as ps512,
        tc.tile_pool(name="ps256", bufs=2, space="PSUM") as ps256,
        tc.tile_pool(name="pssm", bufs=3, space="PSUM") as pssm,
    ):
        # ---------- persistent loads (small/compute-critical first) ------
        adjT, xT, W1, W2, Wg, b1, b2, ga = [], [], [], [], [], [], [], []
        for m in range(M):
            t = pp.tile([128, NDT * S], MM_DT, tag=f"xT{m}", name=f"xT{m}")
            for k in range(NDT):
                nc.scalar.dma_start(
                    out=t[:, k * S:(k + 1) * S],
                    in_=P[f"xTc{m}"][k * 128:(k + 1) * 128, :],
                )
            xT.append(t)

            t = pp.tile([128, NDT * H], MM_DT, tag=f"W1_{m}", name=f"W1_{m}")
            for k in range(NDT):
                nc.scalar.dma_start(
                    out=t[:, k * H:(k + 1) * H],
                    in_=P[f"gc1_W{m}"][k * 128:(k + 1) * 128, :],
                )
            W1.append(t)

            t = pp.tile([128, NFB * H], MM_DT, tag=f"W2_{m}", name=f"W2_{m}")
            for k in range(NFB):
                nc.sync.dma_start(
                    out=t[:, k * H:(k + 1) * H],
                    in_=P[f"gc2_W{m}"][k * 128:(k + 1) * 128, :],
                )
            W2.append(t)

            t = pp.tile([128, NFB * F2], MM_DT, tag=f"Wg_{m}", name=f"Wg_{m}")
            nc.sync.dma_start(
                out=t[:].rearrange("p (t f) -> p t f", t=NFB),
                in_=P[f"gat_W{m}"].rearrange("(t p) f -> p t f", p=128),
            )
            Wg.append(t)

            t = pp.tile([128, NFB], F32, tag=f"b1_{m}", name=f"b1_{m}")
            nc.sync.dma_start(
                out=t[:].rearrange("p (t o) -> p t o", t=NFB),
                in_=P[f"gc1_b{m}"].rearrange("(t p) o -> p t o", p=128),
            )
            b1.append(t)

            t = pp.tile([128, NFB], F32, tag=f"b2_{m}", name=f"b2_{m}")
            nc.sync.dma_start(
                out=t[:].rearrange("p (t o) -> p t o", t=NFB),
                in_=P[f"gc2_b{m}"].rearrange("(t p) o -> p t o", p=128),
            )
            b2.append(t)

            t = pp.tile([128, 2], F32R, tag=f"ga_{m}", name=f"ga_{m}")
            nc.sync.dma_start(
                out=t[:].rearrange("p (t o) -> p t o", t=2),
                in_=P[f"gat_a{m}"].rearrange("(t p) o -> p t o", p=128),
            )
            ga.append(t)

        spW1, spb1, spW2, spb2 = [], [], [], []
        for tag in ("0", "1", "j"):
            key = {"0": ("spW1_0", "spb1_0", "spW2_0", "spb2_0"),
                   "1": ("spW1_1", "spb1_1", "spW2_1", "spb2_1"),
                   "j": ("jpW1", "jpb1", "jpW2", "jpb2")}[tag]
            t = pp.tile([Z, PH], F32, tag=f"spW1{tag}", name=f"spW1{tag}")
            nc.sync.dma_start(out=t[:], in_=P[key[0]][:, :])
            spW1.append(t)
            t = pp.tile([PH, 1], F32, tag=f"spb1{tag}", name=f"spb1{tag}")
            nc.sync.dma_start(out=t[:], in_=P[key[1]][:, :])
            spb1.append(t)
            t = pp.tile([PH, Y], F32, tag=f"spW2{tag}", name=f"spW2{tag}")
            nc.sync.dma_start(out=t[:], in_=P[key[2]][:, :])
            spW2.append(t)
            t = pp.tile([Y, 1], F32, tag=f"spb2{tag}", name=f"spb2{tag}")
            nc.sync.dma_start(out=t[:], in_=P[key[3]][:, :])
            spb2.append(t)

        _dma_engs = [nc.sync, nc.scalar, nc.gpsimd]
        for m in range(M):
            t = pp.tile([128, NJT * N], MM_DT, tag=f"adjT{m}", name=f"adjT{m}")
            for j in range(NJT):
                _dma_engs[(m * NJT + j) % 3].dma_start(
                    out=t[:, j * N:(j + 1) * N],
                    in_=P[f"adjTc{m}"][j * 128:(j + 1) * 128, :],
                )
            adjT.append(t)

        ones_row = pp.tile([1, 128], F32, tag="ones_row")
        nc.vector.memset(ones_row[:], 1.0)
        ones_col = pp.tile([128, 1], F32, tag="ones_col")
        nc.vector.memset(ones_col[:], 1.0)
        ones_row_r = pp.tile([1, 128], F32R, tag="ones_row_r")
        nc.vector.tensor_copy(ones_row_r[:], ones_row[:])
        ones_col_m = pp.tile([128, 1], MM_DT, tag="ones_col_m")
        nc.vector.memset(ones_col_m[:], 1.0)

        # per-modality persistent intermediates
        s1sb = [pp.tile([128, NJT * H], MM_DT, tag=f"s1sb{m}", name=f"s1sb{m}") for m in range(M)]
        s2sb = [pp.tile([128, NJT * H], MM_DT, tag=f"s2sb{m}", name=f"s2sb{m}") for m in range(M)]
        x1T = [pp.tile([128, NFB * S], MM_DT, tag=f"x1T{m}", name=f"x1T{m}") for m in range(M)]
        x2T = [pp.tile([128, NFB * S], MM_DT, tag=f"x2T{m}", name=f"x2T{m}") for m in range(M)]
        hnd = [pp.tile([128, NJT * F2], MM_DT, tag=f"hnd{m}", name=f"hnd{m}") for m in range(M)]
        a2hc = [pp.tile([128, NJT], F32, tag=f"a2hc{m}", name=f"a2hc{m}") for m in range(M)]
        attT = [pp.tile([128, S], F32, tag=f"attT{m}", name=f"attT{m}") for m in range(M)]

        # ---------- stage A: s1 = x_c @ W1 for the core's own 512 nodes ----
        def stage_A(m):
            for it in range(NIB):
                ps = ps256.tile([128, H], F32, tag="psA", bufs=1)
                for k in range(NDT):
                    nc.tensor.matmul(
                        ps[:],
                        xT[m][:, k * S + it * 128: k * S + (it + 1) * 128],
                        W1[m][:, k * H:(k + 1) * H],
                        start=(k == 0), stop=(k == NDT - 1),
                    )
                nc.vector.tensor_copy(s1sb[m][:, it * H:(it + 1) * H], ps[:])

        # ---- aggregation partials: rs_in[ch*H+fb*128, :] = sum_j s adjT ----
        # One chunk-pair (2 ch x 2 fb = 4 PSUM tiles) lands in one [128,2048]
        # SBUF tile and goes out as ONE DMA descriptor ([512,512] DRAM rows).
        def stage_agg_cp(m, ssb, rs_in_t, cp, all_dve=False):
            big = wp.tile([128, 2 * NFB * S], RS_DT, tag="aggcp", bufs=3)
            for ci in range(2):
                ch = cp * 2 + ci
                for fb in range(NFB):
                    psf = ps512.tile([128, S], F32, tag="psAgg", bufs=2)
                    for j in range(NJT):
                        nc.tensor.matmul(
                            psf[:],
                            ssb[:, j * H + fb * 128: j * H + (fb + 1) * 128],
                            adjT[m][:, j * N + ch * S: j * N + (ch + 1) * S],
                            start=(j == 0), stop=(j == NJT - 1),
                        )
                    dst = big[:, (ci * NFB + fb) * S:(ci * NFB + fb + 1) * S]
                    if all_dve or (ci * NFB + fb) % 2 == 0:
                        nc.vector.tensor_copy(dst, psf[:])
                    else:
                        nc.scalar.activation(dst, psf[:], AFT.Copy)
            nc.sync.dma_start(
                out=rs_in_t[cp * 2 * H:(cp + 1) * 2 * H, :]
                    .rearrange("(t p) f -> p t f", p=128),
                in_=big[:].rearrange("p (t f) -> p t f", t=2 * NFB),
            )

        def stage_agg(m, ssb, rs_in_t):
            for cp in range(NCH // 2):
                stage_agg_cp(m, ssb, rs_in_t, cp)

        # Collectives must sit on the Pool queue (the BIR verifier rejects
        # every other engine). The trigger is non-blocking (completion is
        # semaphore-waited by consumers), so Pool copy work still flows.
        def ag_collective(m):
            if NO_COLL:
                return
            nc.gpsimd.collective_compute(
                "AllGather", ALU.bypass, replica_groups=rg,
                ins=[a1h_in[m].ap().opt()], outs=[a1h_out[m].ap().opt()],
            )

        def rs_collective(in_t, out_t):
            if NO_COLL:
                return
            nc.gpsimd.collective_compute(
                "ReduceScatter", ALU.add, replica_groups=rg,
                ins=[in_t.ap().opt()], outs=[out_t.ap().opt()],
            )

        # ---------- stage B: xout_T = prelu(rs_out + b), feature-major -----
        def stage_B(m, rs_out_t, bias, outT):
            raw = wp.tile([128, NFB * S], RS_DT, tag="rsraw", bufs=1)
            nc.sync.dma_start(
                out=raw[:].rearrange("p (t f) -> p t f", t=NFB),
                in_=rs_out_t.ap().rearrange("(t p) f -> p t f", p=128),
            )
            for fb in range(NFB):
                nc.scalar.activation(
                    outT[:, fb * S:(fb + 1) * S], raw[:, fb * S:(fb + 1) * S],
                    AFT.Prelu, bias=bias[:, fb:fb + 1], scale=1.0, alpha=0.25,
                )

        # ---------- stage C: s2 = x1_c @ W2 (own nodes, node-major) --------
        def stage_C(m):
            for it in range(NIB):
                ps = ps256.tile([128, H], F32, tag="psA", bufs=1)
                for fb in range(NFB):
                    nc.tensor.matmul(
                        ps[:],
                        x1T[m][:, fb * S + it * 128: fb * S + (it + 1) * 128],
                        W2[m][:, fb * H:(fb + 1) * H],
                        start=(fb == 0), stop=(fb == NFB - 1),
                    )
                nc.vector.tensor_copy(s2sb[m][:, it * H:(it + 1) * H], ps[:])

        # ---------- stage D: x2T, h (node-major), a1h/a2h; a1h gather ------
        def stage_D(m):
            # h node-major: h[j, g] = sum_f x2[j, f] Wg[f, g]
            for it in range(NIB):
                psh = pssm.tile([128, F2], F32, tag="sm", bufs=2)
                for fb in range(NFB):
                    nc.tensor.matmul(
                        psh[:],
                        x2T[m][:, fb * S + it * 128: fb * S + (it + 1) * 128],
                        Wg[m][:, fb * F2:(fb + 1) * F2],
                        start=(fb == 0), stop=(fb == NFB - 1),
                    )
                nc.vector.tensor_copy(hnd[m][:, it * F2:(it + 1) * F2], psh[:])
            # hT feature-major (for the two a-projections)
            psT = ps512.tile([128, S], F32, tag="psAgg", bufs=2)
            for fb in range(NFB):
                nc.tensor.matmul(
                    psT[:],
                    Wg[m][:, fb * F2:(fb + 1) * F2],
                    x2T[m][:, fb * S:(fb + 1) * S],
                    start=(fb == 0), stop=(fb == NFB - 1),
                )
            hT = wp.tile([128, S], F32R, tag="hTsb", bufs=1)
            nc.vector.tensor_copy(hT[:], psT[:])
            psa1 = pssm.tile([1, S], F32, tag="sm", bufs=2)
            nc.tensor.matmul(psa1[:], ga[m][:, 0:1], hT[:], start=True, stop=True)
            psa2 = pssm.tile([1, S], F32, tag="sm", bufs=2)
            nc.tensor.matmul(psa2[:], ga[m][:, 1:2], hT[:], start=True, stop=True)
            arow1 = smp.tile([1, S], F32R, tag="arow", bufs=2)
            nc.vector.tensor_copy(arow1[:], psa1[:])
            arow2 = smp.tile([1, S], F32, tag="arow", bufs=2)
            nc.vector.tensor_copy(arow2[:], psa2[:])
            # a2h as per-partition bias columns [128, NJT], via DRAM bounce
            nc.sync.dma_start(
                out=a2h_d[m].ap().rearrange("(t p) o -> p (t o)", p=1),
                in_=arow2[:],
            )
            nc.sync.dma_start(
                out=a2hc[m][:],
                in_=a2h_d[m].ap().rearrange("(t p) o -> p (t o)", p=128),
            )
            # a1h slice to DRAM for the per-mod AllGather
            nc.sync.dma_start(
                out=a1h_in[m].ap().rearrange("(t p) o -> p (t o)", p=1),
                in_=arow1[:],
            )

        # ---------- stage F: masked attention partials over own j ----------
        def stage_F_pre(m):
            # all 8 a1h chunks in one DMA, on partition 0 (matmul moving
            # operands must start at partition 0/32/64)
            a1call = smp.tile([1, N], F32R, tag="a1call", bufs=1,
                              name=f"a1call{m}")
            nc.sync.dma_start(
                out=a1call[:],
                in_=a1h_out[m].ap().rearrange("(t p) o -> p (t o)", p=1),
            )
            return a1call

        def stage_F_ch(m, ch, a1call, par):
            psB = ps512.tile([128, S], F32, tag="psB", bufs=1)
            nc.tensor.matmul(
                psB[:], ones_row_r[:], a1call[:, ch * S:(ch + 1) * S],
                start=True, stop=True,
            )
            psO = ps512.tile([128, S], F32, tag="psO", bufs=2)
            psd = pssm.tile([1, S], F32, tag="sm", bufs=2)
            # l needs one ACT op per j (per-partition bias differs), but exp
            # and the mask-mult batch all 4 j-tiles into single wide ops,
            # amortizing the per-instruction access bubble.
            l_all = wp.tile([128, NJT * S], MM_DT, tag="att_l", bufs=2)
            for j in range(NJT):
                nc.scalar.activation(
                    l_all[:, j * S:(j + 1) * S], psB[:], AFT.Prelu,
                    bias=a2hc[m][:, j:j + 1], scale=1.0, alpha=0.25,
                )
            p_all = wp.tile([128, NJT * S], MM_DT, tag="att_p", bufs=2)
            nc.scalar.activation(p_all[:], l_all[:], AFT.Exp)
            pm_all = pmp.tile([128, NJT * S], MM_DT, tag="att_pm", bufs=3)
            if os.environ.get("K_PM_POOL", "0") == "1":
                # Pool rejects scalar_tensor_tensor; mask-gen + mult as two
                # Pool-legal ops (tensor_scalar is_gt, then tensor_tensor).
                m01 = pmp.tile([128, NJT * S], MM_DT, tag="att_m01", bufs=2)
                nc.gpsimd.tensor_scalar(
                    out=m01[:].rearrange("p (j i) -> p j i", j=NJT),
                    in0=adjT[m][:].rearrange("p (j n) -> p j n", j=NJT)
                        [:, :, ch * S:(ch + 1) * S],
                    scalar1=0.0, scalar2=None, op0=ALU.is_gt,
                )
                nc.gpsimd.tensor_tensor(
                    out=pm_all[:], in0=m01[:], in1=p_all[:], op=ALU.mult,
                )
            else:
                nc.vector.scalar_tensor_tensor(
                    out=pm_all[:].rearrange("p (j i) -> p j i", j=NJT),
                    in0=adjT[m][:].rearrange("p (j n) -> p j n", j=NJT)
                        [:, :, ch * S:(ch + 1) * S],
                    scalar=0.0,
                    in1=p_all[:].rearrange("p (j i) -> p j i", j=NJT),
                    op0=ALU.is_gt, op1=ALU.mult,
                )
            for j in range(NJT):
                nc.tensor.matmul(
                    psO[:], hnd[m][:, j * F2:(j + 1) * F2],
                    pm_all[:, j * S:(j + 1) * S],
                    start=(j == 0), stop=(j == NJT - 1),
                )
                nc.tensor.matmul(
                    psd[:], ones_col_m[:], pm_all[:, j * S:(j + 1) * S],
                    start=(j == 0), stop=(j == NJT - 1),
                )
            ob = wp.tile([128, S], RS_DT, tag="att_ob", bufs=2)
            nc.vector.tensor_copy(ob[:], psO[:])
            dn = smp.tile([1, S], RS_DT, tag="att_dn", bufs=2)
            nc.vector.tensor_copy(dn[:], psd[:])
            base = ch * M * ARS + m * ARS
            nc.sync.dma_start(
                out=rsA_in[par].ap()[base: base + F2, :], in_=ob[:])
            nc.sync.dma_start(
                out=rsA_in[par].ap()[base + F2: base + F2 + 1, :], in_=dn[:])

        # ---------- stage E: attention epilogue -> attT (feature-major) ----
        def stage_E(m, par):
            araw = wp.tile([128, S], RS_DT, tag="ep_raw", bufs=2)
            nc.sync.dma_start(
                out=araw[:], in_=rsA_out[par].ap()[m * ARS: m * ARS + F2, :])
            drw = smp.tile([1, S], RS_DT, tag="ep_draw", bufs=2)
            nc.sync.dma_start(
                out=drw[:],
                in_=rsA_out[par].ap()[m * ARS + F2: m * ARS + F2 + 1, :])
            rec = smp.tile([1, S], F32, tag="ep_rec", bufs=2)
            nc.vector.reciprocal(rec[:], drw[:])
            psR = ps512.tile([128, S], F32, tag="psB", bufs=1)
            nc.tensor.matmul(psR[:], ones_row[:], rec[:], start=True, stop=True)
            sc = wp.tile([128, S], F32, tag="ep_sc", bufs=2)
            nc.vector.tensor_tensor(out=sc[:], in0=araw[:], in1=psR[:], op=ALU.mult)
            nc.scalar.activation(attT[m][:], sc[:], AFT.Prelu, alpha=0.25)

        # ---------- stage G: joint PoE + predictors ----------
        def predictor(zT, which, out_row):
            psa = ps512.tile([128, S], F32, tag="psB", bufs=1)
            nc.tensor.matmul(psa[:], spW1[which][:], zT, start=True, stop=True)
            aT = wp.tile([128, S], F32, tag="ep_sc", bufs=2)
            nc.scalar.activation(
                aT[:], psa[:], AFT.Prelu, bias=spb1[which][:, 0:1], scale=1.0,
                alpha=0.25,
            )
            pslg = pssm.tile([Y, S], F32, tag="sm", bufs=2)
            nc.tensor.matmul(pslg[:], spW2[which][:], aT[:], start=True, stop=True)
            ex = smp.tile([Y, S], F32, tag="sm")
            nc.scalar.activation(ex[:], pslg[:], AFT.Exp, bias=spb2[which][:, 0:1], scale=1.0)
            pssum = pssm.tile([1, S], F32, tag="sm", bufs=2)
            nc.tensor.matmul(pssum[:], ones_col[0:Y, 0:1], ex[:], start=True, stop=True)
            rs = smp.tile([1, S], F32, tag="sm")
            nc.vector.reciprocal(rs[:], pssum[:])
            psrb = pssm.tile([Y, S], F32, tag="sm", bufs=2)
            nc.tensor.matmul(psrb[:], ones_row[0:1, 0:Y], rs[:], start=True, stop=True)
            rb = smp.tile([Y, S], F32, tag="sm")
            nc.vector.tensor_copy(rb[:], psrb[:])
            yT = smp.tile([Y, S], F32, tag="sm")
            nc.vector.tensor_tensor(out=yT[:], in0=ex[:], in1=rb[:], op=ALU.mult)
            nc.sync.dma_start(out=P["outT"][out_row:out_row + Y, :], in_=yT[:])

        def stage_G():
            Msb = []
            for m in range(M):
                t = smp.tile([Z, S], F32, tag="sm", name=f"Msb{m}")
                nc.sync.dma_start(
                    out=t[:],
                    in_=P["maskT"][0:1, m * S:(m + 1) * S].to_broadcast((Z, S)),
                )
                Msb.append(t)
            pmm = []
            for m in range(M):
                e = smp.tile([Z, S], F32, tag="sm", name=f"poe_e{m}")
                nc.scalar.activation(e[:], attT[m][Z:2 * Z, :], AFT.Exp)
                pr = smp.tile([Z, S], F32, tag="sm", name=f"poe_pr{m}")
                nc.vector.reciprocal(pr[:], e[:])
                pmt = smp.tile([Z, S], F32, tag="sm", name=f"poe_pm{m}")
                nc.vector.tensor_tensor(out=pmt[:], in0=pr[:], in1=Msb[m][:], op=ALU.mult)
                pmm.append(pmt)
            tmp = smp.tile([Z, S], F32, tag="sm")
            nc.vector.scalar_tensor_tensor(
                out=tmp[:], in0=pmm[0][:], scalar=1.0, in1=pmm[1][:],
                op0=ALU.add, op1=ALU.add,
            )
            jv = smp.tile([Z, S], F32, tag="sm")
            nc.vector.reciprocal(jv[:], tmp[:])
            n0 = smp.tile([Z, S], F32, tag="sm")
            nc.vector.tensor_tensor(out=n0[:], in0=pmm[0][:], in1=attT[0][0:Z, :], op=ALU.mult)
            n1 = smp.tile([Z, S], F32, tag="sm")
            nc.vector.tensor_tensor(out=n1[:], in0=pmm[1][:], in1=attT[1][0:Z, :], op=ALU.mult)
            nsum = smp.tile([Z, S], F32, tag="sm")
            nc.vector.tensor_tensor(out=nsum[:], in0=n0[:], in1=n1[:], op=ALU.add)
            jmu = smp.tile([Z, S], F32, tag="sm")
            nc.vector.tensor_tensor(out=jmu[:], in0=jv[:], in1=nsum[:], op=ALU.mult)

            predictor(jmu[:], 2, 0)

        # ---------- emission: software-pipelined over reps ----------------
        # Rep r's epilogue (E/predictors/G, which wait on the attention RS)
        # is emitted AFTER rep r+1's compute so its long-latency waits don't
        # block the next rep's work in the in-order engine queues.
        def epilogue(par):
            stage_E(0, par)
            predictor(attT[0][0:Z, :], 0, Y)
            stage_E(1, par)
            predictor(attT[1][0:Z, :], 1, 2 * Y)
            stage_G()

        # GCN1 for modality m, split into emission pieces so the NEXT rep's
        # GCN1 can interleave into the current rep's attention chunks (fills
        # PE's attention-phase idle slots and fires next-rep RS1 a phase
        # early). Identity reordering at K_REPS=1.
        def gcn1_pieces(m):
            pieces = [lambda m=m: stage_A(m)]
            for cp in range(NCH // 2):
                pieces.append(
                    lambda m=m, cp=cp: stage_agg_cp(m, s1sb[m][:], rs1_in[m],
                                                    cp, all_dve=True))
            pieces.append(
                lambda m=m: rs_collective(rs1_in[m], rs1_out[m]))
            return pieces

        REPS = int(os.environ.get("K_REPS", "1"))
        pending = None
        pre_done = False
        for _rep in range(REPS):
            par = _rep % 2
            if not pre_done:
                for f in gcn1_pieces(0):
                    f()
                for f in gcn1_pieces(1):
                    f()
            stage_B(0, rs1_out[0], b1[0], x1T[0][:])
            stage_C(0)
            stage_agg(0, s2sb[0][:], rs2_in[0])
            rs_collective(rs2_in[0], rs2_out[0])
            stage_B(1, rs1_out[1], b1[1], x1T[1][:])
            stage_C(1)
            stage_agg(1, s2sb[1][:], rs2_in[1])
            rs_collective(rs2_in[1], rs2_out[1])
            stage_B(0, rs2_out[0], b2[0], x2T[0][:])
            stage_D(0)
            ag_collective(0)
            stage_B(1, rs2_out[1], b2[1], x2T[1][:])
            stage_D(1)
            ag_collective(1)
            last = (_rep == REPS - 1)
            nxt0 = [] if last else gcn1_pieces(0)
            nxt1 = [] if last else gcn1_pieces(1)
            a1call0 = stage_F_pre(0)
            for ch in range(NCH):
                stage_F_ch(0, ch, a1call0, par)
                if ch < len(nxt0):
                    nxt0[ch]()
            a1call1 = stage_F_pre(1)
            for ch in range(NCH):
                stage_F_ch(1, ch, a1call1, par)
                if ch < len(nxt1):
                    nxt1[ch]()
            rs_collective(rsA_in[par], rsA_out[par])
            if pending is not None:
                epilogue(pending)
            pending = par
            pre_done = not last
        epilogue(pending)


def _build_nc():
    nc = bacc.Bacc("TRN2", target_bir_lowering=False, debug=False,
                   num_devices=NCORES)
    P = {}
    for m in range(M):
        P[f"adjTc{m}"] = nc.dram_tensor(f"adjTc{m}", [S, N], MM_DT, kind="ExternalInput").ap()
        P[f"xTc{m}"] = nc.dram_tensor(f"xTc{m}", [DP, S], MM_DT, kind="ExternalInput").ap()
        P[f"gc1_W{m}"] = nc.dram_tensor(f"gc1_W{m}", [DP, H], MM_DT, kind="ExternalInput").ap()
        P[f"gc1_b{m}"] = nc.dram_tensor(f"gc1_b{m}", [H, 1], F32, kind="ExternalInput").ap()
        P[f"gc2_W{m}"] = nc.dram_tensor(f"gc2_W{m}", [H, H], MM_DT, kind="ExternalInput").ap()
        P[f"gc2_b{m}"] = nc.dram_tensor(f"gc2_b{m}", [H, 1], F32, kind="ExternalInput").ap()
        P[f"gat_W{m}"] = nc.dram_tensor(f"gat_W{m}", [H, F2], MM_DT, kind="ExternalInput").ap()
        P[f"gat_a{m}"] = nc.dram_tensor(f"gat_a{m}", [2 * F2, 1], F32R, kind="ExternalInput").ap()
        P[f"spW1_{m}"] = nc.dram_tensor(f"spW1_{m}", [Z, PH], F32, kind="ExternalInput").ap()
        P[f"spb1_{m}"] = nc.dram_tensor(f"spb1_{m}", [PH, 1], F32, kind="ExternalInput").ap()
        P[f"spW2_{m}"] = nc.dram_tensor(f"spW2_{m}", [PH, Y], F32, kind="ExternalInput").ap()
        P[f"spb2_{m}"] = nc.dram_tensor(f"spb2_{m}", [Y, 1], F32, kind="ExternalInput").ap()
    P["jpW1"] = nc.dram_tensor("jpW1", [Z, PH], F32, kind="ExternalInput").ap()
    P["jpb1"] = nc.dram_tensor("jpb1", [PH, 1], F32, kind="ExternalInput").ap()
    P["jpW2"] = nc.dram_tensor("jpW2", [PH, Y], F32, kind="ExternalInput").ap()
    P["jpb2"] = nc.dram_tensor("jpb2", [Y, 1], F32, kind="ExternalInput").ap()
    P["maskT"] = nc.dram_tensor("maskT", [1, M * S], F32, kind="ExternalInput").ap()
    P["outT"] = nc.dram_tensor("outT", [3 * Y, S], F32, kind="ExternalOutput").ap()

    with tile.TileContext(nc) as tc:
        _emit(nc, tc, P)
    nc.compile()
    return nc


@functools.lru_cache(maxsize=1)
def _get_compiled():
    nc = _build_nc()
    nc.m = get_hw_module(nc.m)
    return nc


def _mm_np(a):
    if MM_BF16:
        import ml_dtypes
        return np.ascontiguousarray(a.astype(ml_dtypes.bfloat16))
    return np.ascontiguousarray(np.asarray(a, np.float32))


def _shard_inputs(inputs):
    f = np.float32
    in_maps = []
    pad_w = []
    for m in range(M):
        w = np.zeros((DP, H), f)
        w[:D, :] = inputs[f"gc1_W{m}"]
        pad_w.append(_mm_np(w))
    for c in range(NCORES):
        r0, r1 = c * S, (c + 1) * S
        im = {}
        for m in range(M):
            im[f"adjTc{m}"] = _mm_np(np.asarray(inputs[f"adj{m}"], f)[:, r0:r1].T)
            xp = np.zeros((DP, S), f)
            xp[:D, :] = np.asarray(inputs[f"x{m}"], f)[r0:r1, :].T
            im[f"xTc{m}"] = _mm_np(xp)
            im[f"gc1_W{m}"] = pad_w[m]
            im[f"gc1_b{m}"] = np.asarray(inputs[f"gc1_b{m}"], f).reshape(H, 1)
            im[f"gc2_W{m}"] = _mm_np(np.asarray(inputs[f"gc2_W{m}"], f))
            im[f"gc2_b{m}"] = np.asarray(inputs[f"gc2_b{m}"], f).reshape(H, 1)
            im[f"gat_W{m}"] = _mm_np(np.asarray(inputs[f"gat_W{m}"], f))
            im[f"gat_a{m}"] = np.ascontiguousarray(np.asarray(inputs[f"gat_a{m}"], f))
            im[f"spW1_{m}"] = np.ascontiguousarray(np.asarray(inputs[f"spW1_{m}"], f))
            im[f"spb1_{m}"] = np.asarray(inputs[f"spb1_{m}"], f).reshape(PH, 1)
            im[f"spW2_{m}"] = np.ascontiguousarray(np.asarray(inputs[f"spW2_{m}"], f))
            im[f"spb2_{m}"] = np.asarray(inputs[f"spb2_{m}"], f).reshape(Y, 1)
        im["jpW1"] = np.ascontiguousarray(np.asarray(inputs["jpW1"], f))
        im["jpb1"] = np.asarray(inputs["jpb1"], f).reshape(PH, 1)
        im["jpW2"] = np.ascontiguousarray(np.asarray(inputs["jpW2"], f))
        im["jpb2"] = np.asarray(inputs["jpb2"], f).reshape(Y, 1)
        im["maskT"] = np.ascontiguousarray(
            np.asarray(inputs["mask"], f)[r0:r1, :].T.reshape(1, M * S))
        in_maps.append(im)
    return in_maps


def run(inputs, trace=False):
    nc = _get_compiled()
    in_maps = _shard_inputs(inputs)
    res = run_bass_kernel_spmd(nc, in_maps, list(range(NCORES)), trace=trace)
    out = np.zeros((N, 3 * Y), np.float32)
    for c in range(NCORES):
        out[c * S:(c + 1) * S, :] = res.results[c]["outT"].T
    return out, res


def kernel(**inputs):
    out, _ = run(inputs)
    return out


# revision 37
# speedup vs baseline: 1.0609x; 1.0609x over previous
"""Trainium2 Bass kernel for nn_GCFNN (2-modality GCN+GAT VAE-ish net).

v6 strategy (column-sharded adjacency + software-pipelined epilogue):
- Column-shard the adjacency: core c holds adjTc = adj[:, cS:(c+1)S].T
  ([512 own-j, 4096 all-i], j on partitions) resident in SBUF. Each GCN layer
  computes the support for the core's own 512 nodes locally, then its partial
  aggregation over all 4096 output rows, combined with a per-layer per-mod
  ReduceScatter(add) (0.5 MB out/core instead of a 4 MB AllGather).
- GAT attention stays column-sharded: logits need only a per-mod [N]-vector
  AllGather of a1h; exp/mask/att@h partials plus a ones-row denominator go
  through ONE merged (both-modality) attention ReduceScatter.
- Engine balance: PSUM eviction copies alternate DVE/ACT; the mask-multiply
  is a single 3-D strided DVE op per chunk; exp batches all 4 j-tiles into
  one wide ACT op. (Pool/GPSIMD is compute-useless here: it cannot touch
  PSUM, rejects TensorScalarPtr, and real-HW op launch costs ~6 us.)
- Software pipelining: rep r's epilogue (E/predictors/PoE, which wait on the
  attention RS) is emitted after rep r+1's compute so its long-latency waits
  do not stall the next rep's work in the in-order engine queues; rsA DRAM is
  double-buffered by rep parity. For the real K_REPS=1 build this is a no-op
  reordering. Collectives (Pool queue, the only legal engine; max 2 in
  flight) are ordered so the last trigger of a rep is preceded by cheap AGs,
  reopening the 2-deep trigger window early for the next rep.
- DMA batching: RS-input writes move 4 PSUM tiles per descriptor; rs_out and
  a1h reads are single descriptors.
- Steady-state pipelining: the next rep's GCN1 (supports + aggregation +
  RS1 triggers) is emitted interleaved into the current rep's attention
  chunks (PE fill + early RS1 fire); streaming rings for the RS-input
  staging tile and the attention mask product are 3 deep.
Measured (slope method, see test.py): ~212 us/forward steady-state on the
8-core ring (v3 baseline: ~277 us; wall-clock prints are dominated by a
~8.6 ms axon RPC dispatch floor and do not reflect device time).
"""

import functools
import os
import sys

import numpy as np

if "/opt/trn_rl_repo" not in sys.path:
    sys.path.insert(0, "/opt/trn_rl_repo")

import concourse.bacc as bacc
import concourse.mybir as mybir
import concourse.tile as tile
from concourse.bass_interp import get_hw_module
from concourse.bass_utils import run_bass_kernel_spmd

N, D, H, F2, Z, Y, PH, M = 4096, 400, 256, 128, 64, 5, 128, 2
NCORES = 8
S = N // NCORES          # 512 rows per core
NJT = S // 128           # 4 own j-tiles
NIB = S // 128           # 4 own i-tiles
DP = 512                 # D padded to 4*128
NDT = DP // 128          # 4 d-tiles
NFB = H // 128           # 2 feature blocks
NCH = N // S             # 8 aggregation i-chunks of 512
ARS = F2 + 2             # att RS block rows: 128 out + 1 den + 1 pad
EPS = 1e-8

F32 = mybir.dt.float32
F32R = mybir.dt.float32r
BF16 = mybir.dt.bfloat16

MM_BF16 = os.environ.get("K_MM_BF16", "1") == "1"   # bf16 adj/x/W matmuls
MM_DT = BF16 if MM_BF16 else F32R
RS_DT = BF16
AFT = mybir.ActivationFunctionType
ALU = mybir.AluOpType

NO_COLL = os.environ.get("K_NO_COLL", "0") == "1"


def _emit(nc, tc, P):
    """Emit the whole per-core program. P = dict of DRAM param APs."""
    rs1_in, rs1_out, rs2_in, rs2_out = [], [], [], []
    a1h_in, a1h_out, rsA_in, rsA_out = [], [], [], []
    a2h_d = []
    for m in range(M):
        rs1_in.append(nc.dram_tensor(f"rs1_in{m}", [NCH * H, S], RS_DT))
        rs1_out.append(nc.dram_tensor(f"rs1_out{m}", [H, S], RS_DT))
        rs2_in.append(nc.dram_tensor(f"rs2_in{m}", [NCH * H, S], RS_DT))
        rs2_out.append(nc.dram_tensor(f"rs2_out{m}", [H, S], RS_DT))
        a1h_in.append(nc.dram_tensor(f"a1h_in{m}", [S, 1], F32R))
        a1h_out.append(nc.dram_tensor(f"a1h_out{m}", [N, 1], F32R,
                                      addr_space="Shared"))
        a2h_d.append(nc.dram_tensor(f"a2h_d{m}", [S, 1], F32))
    # double-buffered (by rep parity) so the software-pipelined epilogue of
    # rep r-1 can read while rep r's collective writes the other buffer
    rsA_in = [nc.dram_tensor(f"rsA_in{p}", [NCH * M * ARS, S], RS_DT)
              for p in range(2)]
    rsA_out = [nc.dram_tensor(f"rsA_out{p}", [M * ARS, S], RS_DT)
               for p in range(2)]

    rg = [list(range(NCORES))]

    with (
        tc.tile_pool(name="persist", bufs=1) as pp,
        tc.tile_pool(name="stream", bufs=4) as sp,
        tc.tile_pool(name="work", bufs=3) as wp,
        tc.tile_pool(name="pmpool", bufs=3) as pmp,
        tc.tile_pool(name="small", bufs=4) as smp,
        tc.tile_pool(name="ps512", bufs=4, space="PSUM") as ps512,
        tc.tile_pool(name="ps256", bufs=2, space="PSUM") as ps256,
        tc.tile_pool(name="pssm", bufs=3, space="PSUM") as pssm,
    ):
        # ---------- persistent loads (small/compute-critical first) ------
        adjT, xT, W1, W2, Wg, b1, b2, ga = [], [], [], [], [], [], [], []
        for m in range(M):
            t = pp.tile([128, NDT * S], MM_DT, tag=f"xT{m}", name=f"xT{m}")
            for k in range(NDT):
                nc.scalar.dma_start(
                    out=t[:, k * S:(k + 1) * S],
                    in_=P[f"xTc{m}"][k * 128:(k + 1) * 128, :],
                )
            xT.append(t)

            t = pp.tile([128, NDT * H], MM_DT, tag=f"W1_{m}", name=f"W1_{m}")
            for k in range(NDT):
                nc.scalar.dma_start(
                    out=t[:, k * H:(k + 1) * H],
                    in_=P[f"gc1_W{m}"][k * 128:(k + 1) * 128, :],
                )
            W1.append(t)

            t = pp.tile([128, NFB * H], MM_DT, tag=f"W2_{m}", name=f"W2_{m}")
            for k in range(NFB):
                nc.sync.dma_start(
                    out=t[:, k * H:(k + 1) * H],
                    in_=P[f"gc2_W{m}"][k * 128:(k + 1) * 128, :],
                )
            W2.append(t)

            t = pp.tile([128, NFB * F2], MM_DT, tag=f"Wg_{m}", name=f"Wg_{m}")
            nc.sync.dma_start(
                out=t[:].rearrange("p (t f) -> p t f", t=NFB),
                in_=P[f"gat_W{m}"].rearrange("(t p) f -> p t f", p=128),
            )
            Wg.append(t)

            t = pp.tile([128, NFB], F32, tag=f"b1_{m}", name=f"b1_{m}")
            nc.sync.dma_start(
                out=t[:].rearrange("p (t o) -> p t o", t=NFB),
                in_=P[f"gc1_b{m}"].rearrange("(t p) o -> p t o", p=128),
            )
            b1.append(t)

            t = pp.tile([128, NFB], F32, tag=f"b2_{m}", name=f"b2_{m}")
            nc.sync.dma_start(
                out=t[:].rearrange("p (t o) -> p t o", t=NFB),
                in_=P[f"gc2_b{m}"].rearrange("(t p) o -> p t o", p=128),
            )
            b2.append(t)

            t = pp.tile([128, NFB * 2], MM_DT, tag=f"wa_{m}", name=f"wa_{m}")
            nc.sync.dma_start(
                out=t[:].rearrange("p (t o) -> p t o", t=NFB),
                in_=P[f"wa{m}"].rearrange("(t p) o -> p t o", p=128),
            )
            ga.append(t)

        spW1, spb1, spW2, spb2 = [], [], [], []
        for tag in ("0", "1", "j"):
            key = {"0": ("spW1_0", "spb1_0", "spW2_0", "spb2_0"),
                   "1": ("spW1_1", "spb1_1", "spW2_1", "spb2_1"),
                   "j": ("jpW1", "jpb1", "jpW2", "jpb2")}[tag]
            t = pp.tile([Z, PH], F32, tag=f"spW1{tag}", name=f"spW1{tag}")
            nc.sync.dma_start(out=t[:], in_=P[key[0]][:, :])
            spW1.append(t)
            t = pp.tile([PH, 1], F32, tag=f"spb1{tag}", name=f"spb1{tag}")
            nc.sync.dma_start(out=t[:], in_=P[key[1]][:, :])
            spb1.append(t)
            t = pp.tile([PH, Y], F32, tag=f"spW2{tag}", name=f"spW2{tag}")
            nc.sync.dma_start(out=t[:], in_=P[key[2]][:, :])
            spW2.append(t)
            t = pp.tile([Y, 1], F32, tag=f"spb2{tag}", name=f"spb2{tag}")
            nc.sync.dma_start(out=t[:], in_=P[key[3]][:, :])
            spb2.append(t)

        _dma_engs = [nc.sync, nc.scalar, nc.gpsimd]
        for m in range(M):
            t = pp.tile([128, NJT * N], MM_DT, tag=f"adjT{m}", name=f"adjT{m}")
            for j in range(NJT):
                _dma_engs[(m * NJT + j) % 3].dma_start(
                    out=t[:, j * N:(j + 1) * N],
                    in_=P[f"adjTc{m}"][j * 128:(j + 1) * 128, :],
                )
            adjT.append(t)

        ones_row = pp.tile([1, 128], F32, tag="ones_row")
        nc.vector.memset(ones_row[:], 1.0)
        ones_col = pp.tile([128, 1], F32, tag="ones_col")
        nc.vector.memset(ones_col[:], 1.0)
        ones_row_r = pp.tile([1, 128], F32R, tag="ones_row_r")
        nc.vector.tensor_copy(ones_row_r[:], ones_row[:])
        ones_col_m = pp.tile([128, 1], MM_DT, tag="ones_col_m")
        nc.vector.memset(ones_col_m[:], 1.0)

        # per-modality persistent intermediates
        s1sb = [pp.tile([128, NJT * H], MM_DT, tag=f"s1sb{m}", name=f"s1sb{m}") for m in range(M)]
        s2sb = [pp.tile([128, NJT * H], MM_DT, tag=f"s2sb{m}", name=f"s2sb{m}") for m in range(M)]
        x1T = [pp.tile([128, NFB * S], MM_DT, tag=f"x1T{m}", name=f"x1T{m}") for m in range(M)]
        x2T = [pp.tile([128, NFB * S], MM_DT, tag=f"x2T{m}", name=f"x2T{m}") for m in range(M)]
        hnd = [pp.tile([128, NJT * F2], MM_DT, tag=f"hnd{m}", name=f"hnd{m}") for m in range(M)]
        a2hc = [pp.tile([128, NJT], F32, tag=f"a2hc{m}", name=f"a2hc{m}") for m in range(M)]
        attT = [pp.tile([128, S], F32, tag=f"attT{m}", name=f"attT{m}") for m in range(M)]

        # ---------- stage A: s1 = x_c @ W1 for the core's own 512 nodes ----
        def stage_A(m):
            for it in range(NIB):
                ps = ps256.tile([128, H], F32, tag="psA", bufs=1)
                for k in range(NDT):
                    nc.tensor.matmul(
                        ps[:],
                        xT[m][:, k * S + it * 128: k * S + (it + 1) * 128],
                        W1[m][:, k * H:(k + 1) * H],
                        start=(k == 0), stop=(k == NDT - 1),
                    )
                nc.vector.tensor_copy(s1sb[m][:, it * H:(it + 1) * H], ps[:])

        # ---- aggregation partials: rs_in[ch*H+fb*128, :] = sum_j s adjT ----
        # One chunk-pair (2 ch x 2 fb = 4 PSUM tiles) lands in one [128,2048]
        # SBUF tile and goes out as ONE DMA descriptor ([512,512] DRAM rows).
        def stage_agg_cp(m, ssb, rs_in_t, cp, all_dve=False):
            big = wp.tile([128, 2 * NFB * S], RS_DT, tag="aggcp", bufs=3)
            for ci in range(2):
                ch = cp * 2 + ci
                for fb in range(NFB):
                    psf = ps512.tile([128, S], F32, tag="psAgg", bufs=2)
                    for j in range(NJT):
                        nc.tensor.matmul(
                            psf[:],
                            ssb[:, j * H + fb * 128: j * H + (fb + 1) * 128],
                            adjT[m][:, j * N + ch * S: j * N + (ch + 1) * S],
                            start=(j == 0), stop=(j == NJT - 1),
                        )
                    dst = big[:, (ci * NFB + fb) * S:(ci * NFB + fb + 1) * S]
                    if all_dve or (ci * NFB + fb) % 2 == 0:
                        nc.vector.tensor_copy(dst, psf[:])
                    else:
                        nc.scalar.activation(dst, psf[:], AFT.Copy)
            nc.sync.dma_start(
                out=rs_in_t[cp * 2 * H:(cp + 1) * 2 * H, :]
                    .rearrange("(t p) f -> p t f", p=128),
                in_=big[:].rearrange("p (t f) -> p t f", t=2 * NFB),
            )

        def stage_agg(m, ssb, rs_in_t):
            for cp in range(NCH // 2):
                stage_agg_cp(m, ssb, rs_in_t, cp)

        # Collectives must sit on the Pool queue (the BIR verifier rejects
        # every other engine). The trigger is non-blocking (completion is
        # semaphore-waited by consumers), so Pool copy work still flows.
        def ag_collective(m):
            if NO_COLL:
                return
            nc.gpsimd.collective_compute(
                "AllGather", ALU.bypass, replica_groups=rg,
                ins=[a1h_in[m].ap().opt()], outs=[a1h_out[m].ap().opt()],
            )

        def rs_collective(in_t, out_t):
            if NO_COLL:
                return
            nc.gpsimd.collective_compute(
                "ReduceScatter", ALU.add, replica_groups=rg,
                ins=[in_t.ap().opt()], outs=[out_t.ap().opt()],
            )

        # ---------- stage B: xout_T = prelu(rs_out + b), feature-major -----
        def stage_B(m, rs_out_t, bias, outT):
            raw = wp.tile([128, NFB * S], RS_DT, tag="rsraw", bufs=1)
            nc.sync.dma_start(
                out=raw[:].rearrange("p (t f) -> p t f", t=NFB),
                in_=rs_out_t.ap().rearrange("(t p) f -> p t f", p=128),
            )
            for fb in range(NFB):
                nc.scalar.activation(
                    outT[:, fb * S:(fb + 1) * S], raw[:, fb * S:(fb + 1) * S],
                    AFT.Prelu, bias=bias[:, fb:fb + 1], scale=1.0, alpha=0.25,
                )

        # ---------- stage C: s2 = x1_c @ W2 (own nodes, node-major) --------
        def stage_C(m):
            for it in range(NIB):
                ps = ps256.tile([128, H], F32, tag="psA", bufs=1)
                for fb in range(NFB):
                    nc.tensor.matmul(
                        ps[:],
                        x1T[m][:, fb * S + it * 128: fb * S + (it + 1) * 128],
                        W2[m][:, fb * H:(fb + 1) * H],
                        start=(fb == 0), stop=(fb == NFB - 1),
                    )
                nc.vector.tensor_copy(s2sb[m][:, it * H:(it + 1) * H], ps[:])

        # ---------- stage D: x2T, h (node-major), a1h/a2h; a1h gather ------
        def stage_D(m):
            # h node-major: h[j, g] = sum_f x2[j, f] Wg[f, g]
            for it in range(NIB):
                psh = pssm.tile([128, F2], F32, tag="sm", bufs=2)
                for fb in range(NFB):
                    nc.tensor.matmul(
                        psh[:],
                        x2T[m][:, fb * S + it * 128: fb * S + (it + 1) * 128],
                        Wg[m][:, fb * F2:(fb + 1) * F2],
                        start=(fb == 0), stop=(fb == NFB - 1),
                    )
                nc.vector.tensor_copy(hnd[m][:, it * F2:(it + 1) * F2], psh[:])
            # a1h/a2h directly from x2T via host-folded w_a = gat_W @ gat_a
            psa1 = pssm.tile([1, S], F32, tag="sm", bufs=2)
            psa2 = pssm.tile([1, S], F32, tag="sm", bufs=2)
            for fb in range(NFB):
                nc.tensor.matmul(
                    psa1[:], ga[m][:, fb * 2: fb * 2 + 1],
                    x2T[m][:, fb * S:(fb + 1) * S],
                    start=(fb == 0), stop=(fb == NFB - 1),
                )
                nc.tensor.matmul(
                    psa2[:], ga[m][:, fb * 2 + 1: fb * 2 + 2],
                    x2T[m][:, fb * S:(fb + 1) * S],
                    start=(fb == 0), stop=(fb == NFB - 1),
                )
            arow1 = smp.tile([1, S], F32R, tag="arow", bufs=2)
            nc.vector.tensor_copy(arow1[:], psa1[:])
            arow2 = smp.tile([1, S], F32, tag="arow", bufs=2)
            nc.vector.tensor_copy(arow2[:], psa2[:])
            # a2h as per-partition bias columns [128, NJT], via DRAM bounce
            nc.sync.dma_start(
                out=a2h_d[m].ap().rearrange("(t p) o -> p (t o)", p=1),
                in_=arow2[:],
            )
            nc.sync.dma_start(
                out=a2hc[m][:],
                in_=a2h_d[m].ap().rearrange("(t p) o -> p (t o)", p=128),
            )
            # a1h slice to DRAM for the per-mod AllGather
            nc.sync.dma_start(
                out=a1h_in[m].ap().rearrange("(t p) o -> p (t o)", p=1),
                in_=arow1[:],
            )

        # ---------- stage F: masked attention partials over own j ----------
        def stage_F_pre(m):
            # all 8 a1h chunks in one DMA, on partition 0 (matmul moving
            # operands must start at partition 0/32/64)
            a1call = smp.tile([1, N], F32R, tag="a1call", bufs=1,
                              name=f"a1call{m}")
            nc.sync.dma_start(
                out=a1call[:],
                in_=a1h_out[m].ap().rearrange("(t p) o -> p (t o)", p=1),
            )
            return a1call

        def stage_F_ch(m, ch, a1call, par):
            psB = ps512.tile([128, S], F32, tag="psB", bufs=1)
            nc.tensor.matmul(
                psB[:], ones_row_r[:], a1call[:, ch * S:(ch + 1) * S],
                start=True, stop=True,
            )
            psO = ps512.tile([128, S], F32, tag="psO", bufs=2)
            psd = pssm.tile([1, S], F32, tag="sm", bufs=2)
            # l needs one ACT op per j (per-partition bias differs), but exp
            # and the mask-mult batch all 4 j-tiles into single wide ops,
            # amortizing the per-instruction access bubble.
            l_all = wp.tile([128, NJT * S], MM_DT, tag="att_l", bufs=2)
            for j in range(NJT):
                nc.scalar.activation(
                    l_all[:, j * S:(j + 1) * S], psB[:], AFT.Prelu,
                    bias=a2hc[m][:, j:j + 1], scale=1.0, alpha=0.25,
                )
            p_all = wp.tile([128, NJT * S], MM_DT, tag="att_p", bufs=2)
            nc.scalar.activation(p_all[:], l_all[:], AFT.Exp)
            pm_all = pmp.tile([128, NJT * S], MM_DT, tag="att_pm", bufs=3)
            if os.environ.get("K_PM_POOL", "0") == "1":
                # Pool rejects scalar_tensor_tensor; mask-gen + mult as two
                # Pool-legal ops (tensor_scalar is_gt, then tensor_tensor).
                m01 = pmp.tile([128, NJT * S], MM_DT, tag="att_m01", bufs=2)
                nc.gpsimd.tensor_scalar(
                    out=m01[:].rearrange("p (j i) -> p j i", j=NJT),
                    in0=adjT[m][:].rearrange("p (j n) -> p j n", j=NJT)
                        [:, :, ch * S:(ch + 1) * S],
                    scalar1=0.0, scalar2=None, op0=ALU.is_gt,
                )
                nc.gpsimd.tensor_tensor(
                    out=pm_all[:], in0=m01[:], in1=p_all[:], op=ALU.mult,
                )
            else:
                nc.vector.scalar_tensor_tensor(
                    out=pm_all[:].rearrange("p (j i) -> p j i", j=NJT),
                    in0=adjT[m][:].rearrange("p (j n) -> p j n", j=NJT)
                        [:, :, ch * S:(ch + 1) * S],
                    scalar=0.0,
                    in1=p_all[:].rearrange("p (j i) -> p j i", j=NJT),
                    op0=ALU.is_gt, op1=ALU.mult,
                )
            for j in range(NJT):
                nc.tensor.matmul(
                    psO[:], hnd[m][:, j * F2:(j + 1) * F2],
                    pm_all[:, j * S:(j + 1) * S],
                    start=(j == 0), stop=(j == NJT - 1),
                )
                nc.tensor.matmul(
                    psd[:], ones_col_m[:], pm_all[:, j * S:(j + 1) * S],
                    start=(j == 0), stop=(j == NJT - 1),
                )
            ob = wp.tile([128, S], RS_DT, tag="att_ob", bufs=2)
            nc.vector.tensor_copy(ob[:], psO[:])
            dn = smp.tile([1, S], RS_DT, tag="att_dn", bufs=2)
            nc.vector.tensor_copy(dn[:], psd[:])
            base = ch * M * ARS + m * ARS
            nc.sync.dma_start(
                out=rsA_in[par].ap()[base: base + F2, :], in_=ob[:])
            nc.sync.dma_start(
                out=rsA_in[par].ap()[base + F2: base + F2 + 1, :], in_=dn[:])

        # ---------- stage E: attention epilogue -> attT (feature-major) ----
        def stage_E(m, par):
            araw = wp.tile([128, S], RS_DT, tag="ep_raw", bufs=2)
            nc.sync.dma_start(
                out=araw[:], in_=rsA_out[par].ap()[m * ARS: m * ARS + F2, :])
            drw = smp.tile([1, S], RS_DT, tag="ep_draw", bufs=2)
            nc.sync.dma_start(
                out=drw[:],
                in_=rsA_out[par].ap()[m * ARS + F2: m * ARS + F2 + 1, :])
            rec = smp.tile([1, S], F32, tag="ep_rec", bufs=2)
            nc.vector.reciprocal(rec[:], drw[:])
            psR = ps512.tile([128, S], F32, tag="psB", bufs=1)
            nc.tensor.matmul(psR[:], ones_row[:], rec[:], start=True, stop=True)
            sc = wp.tile([128, S], F32, tag="ep_sc", bufs=2)
            nc.vector.tensor_tensor(out=sc[:], in0=araw[:], in1=psR[:], op=ALU.mult)
            nc.scalar.activation(attT[m][:], sc[:], AFT.Prelu, alpha=0.25)

        # ---------- stage G: joint PoE + predictors ----------
        def predictor(zT, which, out_row):
            psa = ps512.tile([128, S], F32, tag="psB", bufs=1)
            nc.tensor.matmul(psa[:], spW1[which][:], zT, start=True, stop=True)
            aT = wp.tile([128, S], F32, tag="ep_sc", bufs=2)
            nc.scalar.activation(
                aT[:], psa[:], AFT.Prelu, bias=spb1[which][:, 0:1], scale=1.0,
                alpha=0.25,
            )
            pslg = pssm.tile([Y, S], F32, tag="sm", bufs=2)
            nc.tensor.matmul(pslg[:], spW2[which][:], aT[:], start=True, stop=True)
            ex = smp.tile([Y, S], F32, tag="sm")
            nc.scalar.activation(ex[:], pslg[:], AFT.Exp, bias=spb2[which][:, 0:1], scale=1.0)
            pssum = pssm.tile([1, S], F32, tag="sm", bufs=2)
            nc.tensor.matmul(pssum[:], ones_col[0:Y, 0:1], ex[:], start=True, stop=True)
            rs = smp.tile([1, S], F32, tag="sm")
            nc.vector.reciprocal(rs[:], pssum[:])
            psrb = pssm.tile([Y, S], F32, tag="sm", bufs=2)
            nc.tensor.matmul(psrb[:], ones_row[0:1, 0:Y], rs[:], start=True, stop=True)
            rb = smp.tile([Y, S], F32, tag="sm")
            nc.vector.tensor_copy(rb[:], psrb[:])
            yT = smp.tile([Y, S], F32, tag="sm")
            nc.vector.tensor_tensor(out=yT[:], in0=ex[:], in1=rb[:], op=ALU.mult)
            nc.sync.dma_start(out=P["outT"][out_row:out_row + Y, :], in_=yT[:])

        def stage_G():
            Msb = []
            for m in range(M):
                t = smp.tile([Z, S], F32, tag="sm", name=f"Msb{m}")
                nc.sync.dma_start(
                    out=t[:],
                    in_=P["maskT"][0:1, m * S:(m + 1) * S].to_broadcast((Z, S)),
                )
                Msb.append(t)
            pmm = []
            for m in range(M):
                e = smp.tile([Z, S], F32, tag="sm", name=f"poe_e{m}")
                nc.scalar.activation(e[:], attT[m][Z:2 * Z, :], AFT.Exp)
                pr = smp.tile([Z, S], F32, tag="sm", name=f"poe_pr{m}")
                nc.vector.reciprocal(pr[:], e[:])
                pmt = smp.tile([Z, S], F32, tag="sm", name=f"poe_pm{m}")
                nc.vector.tensor_tensor(out=pmt[:], in0=pr[:], in1=Msb[m][:], op=ALU.mult)
                pmm.append(pmt)
            tmp = smp.tile([Z, S], F32, tag="sm")
            nc.vector.scalar_tensor_tensor(
                out=tmp[:], in0=pmm[0][:], scalar=1.0, in1=pmm[1][:],
                op0=ALU.add, op1=ALU.add,
            )
            jv = smp.tile([Z, S], F32, tag="sm")
            nc.vector.reciprocal(jv[:], tmp[:])
            n0 = smp.tile([Z, S], F32, tag="sm")
            nc.vector.tensor_tensor(out=n0[:], in0=pmm[0][:], in1=attT[0][0:Z, :], op=ALU.mult)
            n1 = smp.tile([Z, S], F32, tag="sm")
            nc.vector.tensor_tensor(out=n1[:], in0=pmm[1][:], in1=attT[1][0:Z, :], op=ALU.mult)
            nsum = smp.tile([Z, S], F32, tag="sm")
            nc.vector.tensor_tensor(out=nsum[:], in0=n0[:], in1=n1[:], op=ALU.add)
            jmu = smp.tile([Z, S], F32, tag="sm")
            nc.vector.tensor_tensor(out=jmu[:], in0=jv[:], in1=nsum[:], op=ALU.mult)

            predictor(jmu[:], 2, 0)

        # ---------- emission: software-pipelined over reps ----------------
        # Rep r's epilogue (E/predictors/G, which wait on the attention RS)
        # is emitted AFTER rep r+1's compute so its long-latency waits don't
        # block the next rep's work in the in-order engine queues.
        def epilogue(par):
            stage_E(0, par)
            predictor(attT[0][0:Z, :], 0, Y)
            stage_E(1, par)
            predictor(attT[1][0:Z, :], 1, 2 * Y)
            stage_G()

        # GCN1 for modality m, split into emission pieces so the NEXT rep's
        # GCN1 can interleave into the current rep's attention chunks (fills
        # PE's attention-phase idle slots and fires next-rep RS1 a phase
        # early). Identity reordering at K_REPS=1.
        def gcn1_pieces(m):
            pieces = [lambda m=m: stage_A(m)]
            for cp in range(NCH // 2):
                pieces.append(
                    lambda m=m, cp=cp: stage_agg_cp(m, s1sb[m][:], rs1_in[m],
                                                    cp, all_dve=True))
            pieces.append(
                lambda m=m: rs_collective(rs1_in[m], rs1_out[m]))
            return pieces

        REPS = int(os.environ.get("K_REPS", "1"))
        pending = None
        pre_done = False
        for _rep in range(REPS):
            par = _rep % 2
            if not pre_done:
                for f in gcn1_pieces(0):
                    f()
                for f in gcn1_pieces(1):
                    f()
            stage_B(0, rs1_out[0], b1[0], x1T[0][:])
            stage_C(0)
            stage_agg(0, s2sb[0][:], rs2_in[0])
            rs_collective(rs2_in[0], rs2_out[0])
            stage_B(1, rs1_out[1], b1[1], x1T[1][:])
            stage_C(1)
            stage_agg(1, s2sb[1][:], rs2_in[1])
            rs_collective(rs2_in[1], rs2_out[1])
            stage_B(0, rs2_out[0], b2[0], x2T[0][:])
            stage_D(0)
            ag_collective(0)
            stage_B(1, rs2_out[1], b2[1], x2T[1][:])
            stage_D(1)
            ag_collective(1)
            last = (_rep == REPS - 1)
            nxt0 = [] if last else gcn1_pieces(0)
            nxt1 = [] if last else gcn1_pieces(1)
            a1call0 = stage_F_pre(0)
            for ch in range(NCH):
                stage_F_ch(0, ch, a1call0, par)
                if ch < len(nxt0):
                    nxt0[ch]()
            a1call1 = stage_F_pre(1)
            for ch in range(NCH):
                stage_F_ch(1, ch, a1call1, par)
                if ch < len(nxt1):
                    nxt1[ch]()
            rs_collective(rsA_in[par], rsA_out[par])
            if pending is not None:
                epilogue(pending)
            pending = par
            pre_done = not last
        epilogue(pending)


def _build_nc():
    nc = bacc.Bacc("TRN2", target_bir_lowering=False, debug=False,
                   num_devices=NCORES)
    P = {}
    for m in range(M):
        P[f"adjTc{m}"] = nc.dram_tensor(f"adjTc{m}", [S, N], MM_DT, kind="ExternalInput").ap()
        P[f"xTc{m}"] = nc.dram_tensor(f"xTc{m}", [DP, S], MM_DT, kind="ExternalInput").ap()
        P[f"gc1_W{m}"] = nc.dram_tensor(f"gc1_W{m}", [DP, H], MM_DT, kind="ExternalInput").ap()
        P[f"gc1_b{m}"] = nc.dram_tensor(f"gc1_b{m}", [H, 1], F32, kind="ExternalInput").ap()
        P[f"gc2_W{m}"] = nc.dram_tensor(f"gc2_W{m}", [H, H], MM_DT, kind="ExternalInput").ap()
        P[f"gc2_b{m}"] = nc.dram_tensor(f"gc2_b{m}", [H, 1], F32, kind="ExternalInput").ap()
        P[f"gat_W{m}"] = nc.dram_tensor(f"gat_W{m}", [H, F2], MM_DT, kind="ExternalInput").ap()
        P[f"wa{m}"] = nc.dram_tensor(f"wa{m}", [H, 2], MM_DT, kind="ExternalInput").ap()
        P[f"spW1_{m}"] = nc.dram_tensor(f"spW1_{m}", [Z, PH], F32, kind="ExternalInput").ap()
        P[f"spb1_{m}"] = nc.dram_tensor(f"spb1_{m}", [PH, 1], F32, kind="ExternalInput").ap()
        P[f"spW2_{m}"] = nc.dram_tensor(f"spW2_{m}", [PH, Y], F32, kind="ExternalInput").ap()
        P[f"spb2_{m}"] = nc.dram_tensor(f"spb2_{m}", [Y, 1], F32, kind="ExternalInput").ap()
    P["jpW1"] = nc.dram_tensor("jpW1", [Z, PH], F32, kind="ExternalInput").ap()
    P["jpb1"] = nc.dram_tensor("jpb1", [PH, 1], F32, kind="ExternalInput").ap()
    P["jpW2"] = nc.dram_tensor("jpW2", [PH, Y], F32, kind="ExternalInput").ap()
    P["jpb2"] = nc.dram_tensor("jpb2", [Y, 1], F32, kind="ExternalInput").ap()
    P["maskT"] = nc.dram_tensor("maskT", [1, M * S], F32, kind="ExternalInput").ap()
    P["outT"] = nc.dram_tensor("outT", [3 * Y, S], F32, kind="ExternalOutput").ap()

    with tile.TileContext(nc) as tc:
        _emit(nc, tc, P)
    nc.compile()
    return nc


@functools.lru_cache(maxsize=1)
def _get_compiled():
    nc = _build_nc()
    nc.m = get_hw_module(nc.m)
    return nc


def _mm_np(a):
    if MM_BF16:
        import ml_dtypes
        return np.ascontiguousarray(a.astype(ml_dtypes.bfloat16))
    return np.ascontiguousarray(np.asarray(a, np.float32))


def _shard_inputs(inputs):
    f = np.float32
    in_maps = []
    pad_w = []
    for m in range(M):
        w = np.zeros((DP, H), f)
        w[:D, :] = inputs[f"gc1_W{m}"]
        pad_w.append(_mm_np(w))
    for c in range(NCORES):
        r0, r1 = c * S, (c + 1) * S
        im = {}
        for m in range(M):
            im[f"adjTc{m}"] = _mm_np(np.asarray(inputs[f"adj{m}"], f)[:, r0:r1].T)
            xp = np.zeros((DP, S), f)
            xp[:D, :] = np.asarray(inputs[f"x{m}"], f)[r0:r1, :].T
            im[f"xTc{m}"] = _mm_np(xp)
            im[f"gc1_W{m}"] = pad_w[m]
            im[f"gc1_b{m}"] = np.asarray(inputs[f"gc1_b{m}"], f).reshape(H, 1)
            im[f"gc2_W{m}"] = _mm_np(np.asarray(inputs[f"gc2_W{m}"], f))
            im[f"gc2_b{m}"] = np.asarray(inputs[f"gc2_b{m}"], f).reshape(H, 1)
            im[f"gat_W{m}"] = _mm_np(np.asarray(inputs[f"gat_W{m}"], f))
            ga_full = np.asarray(inputs[f"gat_a{m}"], f)
            wa = np.asarray(inputs[f"gat_W{m}"], f) @ np.stack(
                [ga_full[:F2, 0], ga_full[F2:, 0]], axis=1)
            im[f"wa{m}"] = _mm_np(wa)
            im[f"spW1_{m}"] = np.ascontiguousarray(np.asarray(inputs[f"spW1_{m}"], f))
            im[f"spb1_{m}"] = np.asarray(inputs[f"spb1_{m}"], f).reshape(PH, 1)
            im[f"spW2_{m}"] = np.ascontiguousarray(np.asarray(inputs[f"spW2_{m}"], f))
            im[f"spb2_{m}"] = np.asarray(inputs[f"spb2_{m}"], f).reshape(Y, 1)
        im["jpW1"] = np.ascontiguousarray(np.asarray(inputs["jpW1"], f))
        im["jpb1"] = np.asarray(inputs["jpb1"], f).reshape(PH, 1)
        im["jpW2"] = np.ascontiguousarray(np.asarray(inputs["jpW2"], f))
        im["jpb2"] = np.asarray(inputs["jpb2"], f).reshape(Y, 1)
        im["maskT"] = np.ascontiguousarray(
            np.asarray(inputs["mask"], f)[r0:r1, :].T.reshape(1, M * S))
        in_maps.append(im)
    return in_maps


def run(inputs, trace=False):
    nc = _get_compiled()
    in_maps = _shard_inputs(inputs)
    res = run_bass_kernel_spmd(nc, in_maps, list(range(NCORES)), trace=trace)
    out = np.zeros((N, 3 * Y), np.float32)
    for c in range(NCORES):
        out[c * S:(c + 1) * S, :] = res.results[c]["outT"].T
    return out, res


def kernel(**inputs):
    out, _ = run(inputs)
    return out


# revision 38
# speedup vs baseline: 1.0625x; 1.0015x over previous
"""Trainium2 Bass kernel for nn_GCFNN (2-modality GCN+GAT VAE-ish net).

v6 strategy (column-sharded adjacency + software-pipelined epilogue):
- Column-shard the adjacency: core c holds adjTc = adj[:, cS:(c+1)S].T
  ([512 own-j, 4096 all-i], j on partitions) resident in SBUF. Each GCN layer
  computes the support for the core's own 512 nodes locally, then its partial
  aggregation over all 4096 output rows, combined with a per-layer per-mod
  ReduceScatter(add) (0.5 MB out/core instead of a 4 MB AllGather).
- GAT attention stays column-sharded: logits need only a per-mod [N]-vector
  AllGather of a1h; exp/mask/att@h partials plus a ones-row denominator go
  through ONE merged (both-modality) attention ReduceScatter.
- Engine balance: PSUM eviction copies alternate DVE/ACT; the mask-multiply
  is a single 3-D strided DVE op per chunk; exp batches all 4 j-tiles into
  one wide ACT op. (Pool/GPSIMD is compute-useless here: it cannot touch
  PSUM, rejects TensorScalarPtr, and real-HW op launch costs ~6 us.)
- Software pipelining: rep r's epilogue (E/predictors/PoE, which wait on the
  attention RS) is emitted after rep r+1's compute so its long-latency waits
  do not stall the next rep's work in the in-order engine queues; rsA DRAM is
  double-buffered by rep parity. For the real K_REPS=1 build this is a no-op
  reordering. Collectives (Pool queue, the only legal engine; max 2 in
  flight) are ordered so the last trigger of a rep is preceded by cheap AGs,
  reopening the 2-deep trigger window early for the next rep.
- DMA batching: RS-input writes move 4 PSUM tiles per descriptor; rs_out and
  a1h reads are single descriptors.
- Steady-state pipelining: the next rep's GCN1 (supports + aggregation +
  RS1 triggers) is emitted interleaved into the current rep's attention
  chunks (PE fill + early RS1 fire); streaming rings for the RS-input
  staging tile and the attention mask product are 3 deep.
Measured (slope method, see test.py): ~212 us/forward steady-state on the
8-core ring (v3 baseline: ~277 us; wall-clock prints are dominated by a
~8.6 ms axon RPC dispatch floor and do not reflect device time).
"""

import functools
import os
import sys

import numpy as np

if "/opt/trn_rl_repo" not in sys.path:
    sys.path.insert(0, "/opt/trn_rl_repo")

import concourse.bacc as bacc
import concourse.mybir as mybir
import concourse.tile as tile
from concourse.bass_interp import get_hw_module
from concourse.bass_utils import run_bass_kernel_spmd

N, D, H, F2, Z, Y, PH, M = 4096, 400, 256, 128, 64, 5, 128, 2
NCORES = 8
S = N // NCORES          # 512 rows per core
NJT = S // 128           # 4 own j-tiles
NIB = S // 128           # 4 own i-tiles
DP = 512                 # D padded to 4*128
NDT = DP // 128          # 4 d-tiles
NFB = H // 128           # 2 feature blocks
NCH = N // S             # 8 aggregation i-chunks of 512
ARS = F2 + 2             # att RS block rows: 128 out + 1 den + 1 pad
EPS = 1e-8

F32 = mybir.dt.float32
F32R = mybir.dt.float32r
BF16 = mybir.dt.bfloat16

MM_BF16 = os.environ.get("K_MM_BF16", "1") == "1"   # bf16 adj/x/W matmuls
MM_DT = BF16 if MM_BF16 else F32R
RS_DT = BF16
AFT = mybir.ActivationFunctionType
ALU = mybir.AluOpType

NO_COLL = os.environ.get("K_NO_COLL", "0") == "1"


def _emit(nc, tc, P):
    """Emit the whole per-core program. P = dict of DRAM param APs."""
    rs1_in, rs1_out, rs2_in, rs2_out = [], [], [], []
    a1h_in, a1h_out, rsA_in, rsA_out = [], [], [], []
    a2h_d = []
    for m in range(M):
        rs1_in.append(nc.dram_tensor(f"rs1_in{m}", [NCH * H, S], RS_DT))
        rs1_out.append(nc.dram_tensor(f"rs1_out{m}", [H, S], RS_DT))
        rs2_in.append(nc.dram_tensor(f"rs2_in{m}", [NCH * H, S], RS_DT))
        rs2_out.append(nc.dram_tensor(f"rs2_out{m}", [H, S], RS_DT))
        a1h_in.append(nc.dram_tensor(f"a1h_in{m}", [S, 1], F32R))
        a1h_out.append(nc.dram_tensor(f"a1h_out{m}", [N, 1], F32R,
                                      addr_space="Shared"))
        a2h_d.append(nc.dram_tensor(f"a2h_d{m}", [S, 1], F32))
    # double-buffered (by rep parity) so the software-pipelined epilogue of
    # rep r-1 can read while rep r's collective writes the other buffer
    rsA_in = [nc.dram_tensor(f"rsA_in{p}", [NCH * M * ARS, S], RS_DT)
              for p in range(2)]
    rsA_out = [nc.dram_tensor(f"rsA_out{p}", [M * ARS, S], RS_DT)
               for p in range(2)]

    rg = [list(range(NCORES))]

    with (
        tc.tile_pool(name="persist", bufs=1) as pp,
        tc.tile_pool(name="stream", bufs=4) as sp,
        tc.tile_pool(name="work", bufs=3) as wp,
        tc.tile_pool(name="pmpool", bufs=3) as pmp,
        tc.tile_pool(name="small", bufs=4) as smp,
        tc.tile_pool(name="ps512", bufs=4, space="PSUM") as ps512,
        tc.tile_pool(name="ps256", bufs=2, space="PSUM") as ps256,
        tc.tile_pool(name="pssm", bufs=3, space="PSUM") as pssm,
    ):
        # ---------- persistent loads (small/compute-critical first) ------
        adjT, xT, W1, W2, Wg, b1, b2, ga = [], [], [], [], [], [], [], []
        for m in range(M):
            t = pp.tile([128, NDT * S], MM_DT, tag=f"xT{m}", name=f"xT{m}")
            for k in range(NDT):
                nc.scalar.dma_start(
                    out=t[:, k * S:(k + 1) * S],
                    in_=P[f"xTc{m}"][k * 128:(k + 1) * 128, :],
                )
            xT.append(t)

            t = pp.tile([128, NDT * H], MM_DT, tag=f"W1_{m}", name=f"W1_{m}")
            for k in range(NDT):
                nc.scalar.dma_start(
                    out=t[:, k * H:(k + 1) * H],
                    in_=P[f"gc1_W{m}"][k * 128:(k + 1) * 128, :],
                )
            W1.append(t)

            t = pp.tile([128, NFB * H], MM_DT, tag=f"W2_{m}", name=f"W2_{m}")
            for k in range(NFB):
                nc.sync.dma_start(
                    out=t[:, k * H:(k + 1) * H],
                    in_=P[f"gc2_W{m}"][k * 128:(k + 1) * 128, :],
                )
            W2.append(t)

            t = pp.tile([128, NFB * F2], MM_DT, tag=f"Wg_{m}", name=f"Wg_{m}")
            nc.sync.dma_start(
                out=t[:].rearrange("p (t f) -> p t f", t=NFB),
                in_=P[f"gat_W{m}"].rearrange("(t p) f -> p t f", p=128),
            )
            Wg.append(t)

            t = pp.tile([128, NFB], F32, tag=f"b1_{m}", name=f"b1_{m}")
            nc.sync.dma_start(
                out=t[:].rearrange("p (t o) -> p t o", t=NFB),
                in_=P[f"gc1_b{m}"].rearrange("(t p) o -> p t o", p=128),
            )
            b1.append(t)

            t = pp.tile([128, NFB], F32, tag=f"b2_{m}", name=f"b2_{m}")
            nc.sync.dma_start(
                out=t[:].rearrange("p (t o) -> p t o", t=NFB),
                in_=P[f"gc2_b{m}"].rearrange("(t p) o -> p t o", p=128),
            )
            b2.append(t)

            t = pp.tile([128, NFB * 2], MM_DT, tag=f"wa_{m}", name=f"wa_{m}")
            nc.sync.dma_start(
                out=t[:].rearrange("p (t o) -> p t o", t=NFB),
                in_=P[f"wa{m}"].rearrange("(t p) o -> p t o", p=128),
            )
            ga.append(t)

        spW1, spb1, spW2, spb2 = [], [], [], []
        for tag in ("0", "1", "j"):
            key = {"0": ("spW1_0", "spb1_0", "spW2_0", "spb2_0"),
                   "1": ("spW1_1", "spb1_1", "spW2_1", "spb2_1"),
                   "j": ("jpW1", "jpb1", "jpW2", "jpb2")}[tag]
            t = pp.tile([Z, PH], F32, tag=f"spW1{tag}", name=f"spW1{tag}")
            nc.sync.dma_start(out=t[:], in_=P[key[0]][:, :])
            spW1.append(t)
            t = pp.tile([PH, 1], F32, tag=f"spb1{tag}", name=f"spb1{tag}")
            nc.sync.dma_start(out=t[:], in_=P[key[1]][:, :])
            spb1.append(t)
            t = pp.tile([PH, Y], F32, tag=f"spW2{tag}", name=f"spW2{tag}")
            nc.sync.dma_start(out=t[:], in_=P[key[2]][:, :])
            spW2.append(t)
            t = pp.tile([Y, 1], F32, tag=f"spb2{tag}", name=f"spb2{tag}")
            nc.sync.dma_start(out=t[:], in_=P[key[3]][:, :])
            spb2.append(t)

        _dma_engs = [nc.sync, nc.scalar, nc.gpsimd]
        for m in range(M):
            t = pp.tile([128, NJT * N], MM_DT, tag=f"adjT{m}", name=f"adjT{m}")
            for j in range(NJT):
                _dma_engs[(m * NJT + j) % 3].dma_start(
                    out=t[:, j * N:(j + 1) * N],
                    in_=P[f"adjTc{m}"][j * 128:(j + 1) * 128, :],
                )
            adjT.append(t)

        ones_row = pp.tile([1, 128], F32, tag="ones_row")
        nc.vector.memset(ones_row[:], 1.0)
        ones_col = pp.tile([128, 1], F32, tag="ones_col")
        nc.vector.memset(ones_col[:], 1.0)
        ones_row_r = pp.tile([1, 128], F32R, tag="ones_row_r")
        nc.vector.tensor_copy(ones_row_r[:], ones_row[:])
        ones_col_m = pp.tile([128, 1], MM_DT, tag="ones_col_m")
        nc.vector.memset(ones_col_m[:], 1.0)

        # per-modality persistent intermediates
        s1sb = [pp.tile([128, NJT * H], MM_DT, tag=f"s1sb{m}", name=f"s1sb{m}") for m in range(M)]
        s2sb = [pp.tile([128, NJT * H], MM_DT, tag=f"s2sb{m}", name=f"s2sb{m}") for m in range(M)]
        x1T = [pp.tile([128, NFB * S], MM_DT, tag=f"x1T{m}", name=f"x1T{m}") for m in range(M)]
        x2T = [pp.tile([128, NFB * S], MM_DT, tag=f"x2T{m}", name=f"x2T{m}") for m in range(M)]
        hnd = [pp.tile([128, NJT * F2], MM_DT, tag=f"hnd{m}", name=f"hnd{m}") for m in range(M)]
        a2hc = [pp.tile([128, NJT], F32, tag=f"a2hc{m}", name=f"a2hc{m}") for m in range(M)]
        attT = [pp.tile([128, S], F32, tag=f"attT{m}", name=f"attT{m}") for m in range(M)]

        # ---------- stage A: s1 = x_c @ W1 for the core's own 512 nodes ----
        def stage_A(m):
            for it in range(NIB):
                ps = ps256.tile([128, H], F32, tag="psA", bufs=1)
                for k in range(NDT):
                    nc.tensor.matmul(
                        ps[:],
                        xT[m][:, k * S + it * 128: k * S + (it + 1) * 128],
                        W1[m][:, k * H:(k + 1) * H],
                        start=(k == 0), stop=(k == NDT - 1),
                    )
                nc.vector.tensor_copy(s1sb[m][:, it * H:(it + 1) * H], ps[:])

        # ---- aggregation partials: rs_in[ch*H+fb*128, :] = sum_j s adjT ----
        # One chunk-pair (2 ch x 2 fb = 4 PSUM tiles) lands in one [128,2048]
        # SBUF tile and goes out as ONE DMA descriptor ([512,512] DRAM rows).
        def stage_agg_cp(m, ssb, rs_in_t, cp, all_dve=False):
            big = wp.tile([128, 2 * NFB * S], RS_DT, tag="aggcp", bufs=3)
            for ci in range(2):
                ch = cp * 2 + ci
                for fb in range(NFB):
                    psf = ps512.tile([128, S], F32, tag="psAgg", bufs=2)
                    for j in range(NJT):
                        nc.tensor.matmul(
                            psf[:],
                            ssb[:, j * H + fb * 128: j * H + (fb + 1) * 128],
                            adjT[m][:, j * N + ch * S: j * N + (ch + 1) * S],
                            start=(j == 0), stop=(j == NJT - 1),
                        )
                    dst = big[:, (ci * NFB + fb) * S:(ci * NFB + fb + 1) * S]
                    if all_dve or (ci * NFB + fb) % 2 == 0:
                        nc.vector.tensor_copy(dst, psf[:])
                    else:
                        nc.scalar.activation(dst, psf[:], AFT.Copy)
            nc.sync.dma_start(
                out=rs_in_t[cp * 2 * H:(cp + 1) * 2 * H, :]
                    .rearrange("(t p) f -> p t f", p=128),
                in_=big[:].rearrange("p (t f) -> p t f", t=2 * NFB),
            )

        def stage_agg(m, ssb, rs_in_t):
            for cp in range(NCH // 2):
                stage_agg_cp(m, ssb, rs_in_t, cp)

        # Collectives must sit on the Pool queue (the BIR verifier rejects
        # every other engine). The trigger is non-blocking (completion is
        # semaphore-waited by consumers), so Pool copy work still flows.
        def ag_collective(m):
            if NO_COLL:
                return
            nc.gpsimd.collective_compute(
                "AllGather", ALU.bypass, replica_groups=rg,
                ins=[a1h_in[m].ap().opt()], outs=[a1h_out[m].ap().opt()],
            )

        def rs_collective(in_t, out_t):
            if NO_COLL:
                return
            nc.gpsimd.collective_compute(
                "ReduceScatter", ALU.add, replica_groups=rg,
                ins=[in_t.ap().opt()], outs=[out_t.ap().opt()],
            )

        # ---------- stage B: xout_T = prelu(rs_out + b), feature-major -----
        def stage_B(m, rs_out_t, bias, outT):
            raw = wp.tile([128, NFB * S], RS_DT, tag="rsraw", bufs=1)
            nc.sync.dma_start(
                out=raw[:].rearrange("p (t f) -> p t f", t=NFB),
                in_=rs_out_t.ap().rearrange("(t p) f -> p t f", p=128),
            )
            for fb in range(NFB):
                nc.scalar.activation(
                    outT[:, fb * S:(fb + 1) * S], raw[:, fb * S:(fb + 1) * S],
                    AFT.Prelu, bias=bias[:, fb:fb + 1], scale=1.0, alpha=0.25,
                )

        # ---------- stage C: s2 = x1_c @ W2 (own nodes, node-major) --------
        def stage_C(m):
            for it in range(NIB):
                ps = ps256.tile([128, H], F32, tag="psA", bufs=1)
                for fb in range(NFB):
                    nc.tensor.matmul(
                        ps[:],
                        x1T[m][:, fb * S + it * 128: fb * S + (it + 1) * 128],
                        W2[m][:, fb * H:(fb + 1) * H],
                        start=(fb == 0), stop=(fb == NFB - 1),
                    )
                nc.vector.tensor_copy(s2sb[m][:, it * H:(it + 1) * H], ps[:])

        # ---------- stage D: x2T, h (node-major), a1h/a2h; a1h gather ------
        def stage_D(m):
            # h node-major: h[j, g] = sum_f x2[j, f] Wg[f, g]
            for it in range(NIB):
                psh = pssm.tile([128, F2], F32, tag="sm", bufs=2)
                for fb in range(NFB):
                    nc.tensor.matmul(
                        psh[:],
                        x2T[m][:, fb * S + it * 128: fb * S + (it + 1) * 128],
                        Wg[m][:, fb * F2:(fb + 1) * F2],
                        start=(fb == 0), stop=(fb == NFB - 1),
                    )
                nc.vector.tensor_copy(hnd[m][:, it * F2:(it + 1) * F2], psh[:])
            # a1h/a2h directly from x2T via host-folded w_a = gat_W @ gat_a
            psa1 = pssm.tile([1, S], F32, tag="sm", bufs=2)
            psa2 = pssm.tile([1, S], F32, tag="sm", bufs=2)
            for fb in range(NFB):
                nc.tensor.matmul(
                    psa1[:], ga[m][:, fb * 2: fb * 2 + 1],
                    x2T[m][:, fb * S:(fb + 1) * S],
                    start=(fb == 0), stop=(fb == NFB - 1),
                )
                nc.tensor.matmul(
                    psa2[:], ga[m][:, fb * 2 + 1: fb * 2 + 2],
                    x2T[m][:, fb * S:(fb + 1) * S],
                    start=(fb == 0), stop=(fb == NFB - 1),
                )
            arow1 = smp.tile([1, S], F32R, tag="arow", bufs=2)
            nc.vector.tensor_copy(arow1[:], psa1[:])
            arow2 = smp.tile([1, S], F32, tag="arow", bufs=2)
            nc.vector.tensor_copy(arow2[:], psa2[:])
            # a2h as per-partition bias columns [128, NJT], via DRAM bounce
            nc.sync.dma_start(
                out=a2h_d[m].ap().rearrange("(t p) o -> p (t o)", p=1),
                in_=arow2[:],
            )
            nc.sync.dma_start(
                out=a2hc[m][:],
                in_=a2h_d[m].ap().rearrange("(t p) o -> p (t o)", p=128),
            )
            # a1h slice to DRAM for the per-mod AllGather
            nc.sync.dma_start(
                out=a1h_in[m].ap().rearrange("(t p) o -> p (t o)", p=1),
                in_=arow1[:],
            )

        # ---------- stage F: masked attention partials over own j ----------
        def stage_F_pre(m):
            # all 8 a1h chunks in one DMA, on partition 0 (matmul moving
            # operands must start at partition 0/32/64)
            a1call = smp.tile([1, N], F32R, tag="a1call", bufs=1,
                              name=f"a1call{m}")
            nc.sync.dma_start(
                out=a1call[:],
                in_=a1h_out[m].ap().rearrange("(t p) o -> p (t o)", p=1),
            )
            return a1call

        def stage_F_ch(m, ch, a1call, par):
            psB = ps512.tile([128, S], F32, tag="psB", bufs=1)
            nc.tensor.matmul(
                psB[:], ones_row_r[:], a1call[:, ch * S:(ch + 1) * S],
                start=True, stop=True,
            )
            psO = ps512.tile([128, S], F32, tag="psO", bufs=2)
            psd = pssm.tile([1, S], F32, tag="sm", bufs=2)
            # l needs one ACT op per j (per-partition bias differs), but exp
            # and the mask-mult batch all 4 j-tiles into single wide ops,
            # amortizing the per-instruction access bubble.
            l_all = wp.tile([128, NJT * S], MM_DT, tag="att_l", bufs=2)
            for j in range(NJT):
                nc.scalar.activation(
                    l_all[:, j * S:(j + 1) * S], psB[:], AFT.Prelu,
                    bias=a2hc[m][:, j:j + 1], scale=1.0, alpha=0.25,
                )
            p_all = wp.tile([128, NJT * S], MM_DT, tag="att_p", bufs=2)
            nc.scalar.activation(p_all[:], l_all[:], AFT.Exp)
            pm_all = pmp.tile([128, NJT * S], MM_DT, tag="att_pm", bufs=4)
            if os.environ.get("K_PM_POOL", "0") == "1":
                # Pool rejects scalar_tensor_tensor; mask-gen + mult as two
                # Pool-legal ops (tensor_scalar is_gt, then tensor_tensor).
                m01 = pmp.tile([128, NJT * S], MM_DT, tag="att_m01", bufs=2)
                nc.gpsimd.tensor_scalar(
                    out=m01[:].rearrange("p (j i) -> p j i", j=NJT),
                    in0=adjT[m][:].rearrange("p (j n) -> p j n", j=NJT)
                        [:, :, ch * S:(ch + 1) * S],
                    scalar1=0.0, scalar2=None, op0=ALU.is_gt,
                )
                nc.gpsimd.tensor_tensor(
                    out=pm_all[:], in0=m01[:], in1=p_all[:], op=ALU.mult,
                )
            else:
                nc.vector.scalar_tensor_tensor(
                    out=pm_all[:].rearrange("p (j i) -> p j i", j=NJT),
                    in0=adjT[m][:].rearrange("p (j n) -> p j n", j=NJT)
                        [:, :, ch * S:(ch + 1) * S],
                    scalar=0.0,
                    in1=p_all[:].rearrange("p (j i) -> p j i", j=NJT),
                    op0=ALU.is_gt, op1=ALU.mult,
                )
            for j in range(NJT):
                nc.tensor.matmul(
                    psO[:], hnd[m][:, j * F2:(j + 1) * F2],
                    pm_all[:, j * S:(j + 1) * S],
                    start=(j == 0), stop=(j == NJT - 1),
                )
                nc.tensor.matmul(
                    psd[:], ones_col_m[:], pm_all[:, j * S:(j + 1) * S],
                    start=(j == 0), stop=(j == NJT - 1),
                )
            ob = wp.tile([128, S], RS_DT, tag="att_ob", bufs=2)
            nc.vector.tensor_copy(ob[:], psO[:])
            dn = smp.tile([1, S], RS_DT, tag="att_dn", bufs=2)
            nc.vector.tensor_copy(dn[:], psd[:])
            base = ch * M * ARS + m * ARS
            nc.sync.dma_start(
                out=rsA_in[par].ap()[base: base + F2, :], in_=ob[:])
            nc.sync.dma_start(
                out=rsA_in[par].ap()[base + F2: base + F2 + 1, :], in_=dn[:])

        # ---------- stage E: attention epilogue -> attT (feature-major) ----
        def stage_E(m, par):
            araw = wp.tile([128, S], RS_DT, tag="ep_raw", bufs=2)
            nc.sync.dma_start(
                out=araw[:], in_=rsA_out[par].ap()[m * ARS: m * ARS + F2, :])
            drw = smp.tile([1, S], RS_DT, tag="ep_draw", bufs=2)
            nc.sync.dma_start(
                out=drw[:],
                in_=rsA_out[par].ap()[m * ARS + F2: m * ARS + F2 + 1, :])
            rec = smp.tile([1, S], F32, tag="ep_rec", bufs=2)
            nc.vector.reciprocal(rec[:], drw[:])
            psR = ps512.tile([128, S], F32, tag="psB", bufs=1)
            nc.tensor.matmul(psR[:], ones_row[:], rec[:], start=True, stop=True)
            sc = wp.tile([128, S], F32, tag="ep_sc", bufs=2)
            nc.vector.tensor_tensor(out=sc[:], in0=araw[:], in1=psR[:], op=ALU.mult)
            nc.scalar.activation(attT[m][:], sc[:], AFT.Prelu, alpha=0.25)

        # ---------- stage G: joint PoE + predictors ----------
        def predictor(zT, which, out_row):
            psa = ps512.tile([128, S], F32, tag="psB", bufs=1)
            nc.tensor.matmul(psa[:], spW1[which][:], zT, start=True, stop=True)
            aT = wp.tile([128, S], F32, tag="ep_sc", bufs=2)
            nc.scalar.activation(
                aT[:], psa[:], AFT.Prelu, bias=spb1[which][:, 0:1], scale=1.0,
                alpha=0.25,
            )
            pslg = pssm.tile([Y, S], F32, tag="sm", bufs=2)
            nc.tensor.matmul(pslg[:], spW2[which][:], aT[:], start=True, stop=True)
            ex = smp.tile([Y, S], F32, tag="sm")
            nc.scalar.activation(ex[:], pslg[:], AFT.Exp, bias=spb2[which][:, 0:1], scale=1.0)
            pssum = pssm.tile([1, S], F32, tag="sm", bufs=2)
            nc.tensor.matmul(pssum[:], ones_col[0:Y, 0:1], ex[:], start=True, stop=True)
            rs = smp.tile([1, S], F32, tag="sm")
            nc.vector.reciprocal(rs[:], pssum[:])
            psrb = pssm.tile([Y, S], F32, tag="sm", bufs=2)
            nc.tensor.matmul(psrb[:], ones_row[0:1, 0:Y], rs[:], start=True, stop=True)
            rb = smp.tile([Y, S], F32, tag="sm")
            nc.vector.tensor_copy(rb[:], psrb[:])
            yT = smp.tile([Y, S], F32, tag="sm")
            nc.vector.tensor_tensor(out=yT[:], in0=ex[:], in1=rb[:], op=ALU.mult)
            nc.sync.dma_start(out=P["outT"][out_row:out_row + Y, :], in_=yT[:])

        def stage_G():
            Msb = []
            for m in range(M):
                t = smp.tile([Z, S], F32, tag="sm", name=f"Msb{m}")
                nc.sync.dma_start(
                    out=t[:],
                    in_=P["maskT"][0:1, m * S:(m + 1) * S].to_broadcast((Z, S)),
                )
                Msb.append(t)
            pmm = []
            for m in range(M):
                e = smp.tile([Z, S], F32, tag="sm", name=f"poe_e{m}")
                nc.scalar.activation(e[:], attT[m][Z:2 * Z, :], AFT.Exp)
                pr = smp.tile([Z, S], F32, tag="sm", name=f"poe_pr{m}")
                nc.vector.reciprocal(pr[:], e[:])
                pmt = smp.tile([Z, S], F32, tag="sm", name=f"poe_pm{m}")
                nc.vector.tensor_tensor(out=pmt[:], in0=pr[:], in1=Msb[m][:], op=ALU.mult)
                pmm.append(pmt)
            tmp = smp.tile([Z, S], F32, tag="sm")
            nc.vector.scalar_tensor_tensor(
                out=tmp[:], in0=pmm[0][:], scalar=1.0, in1=pmm[1][:],
                op0=ALU.add, op1=ALU.add,
            )
            jv = smp.tile([Z, S], F32, tag="sm")
            nc.vector.reciprocal(jv[:], tmp[:])
            n0 = smp.tile([Z, S], F32, tag="sm")
            nc.vector.tensor_tensor(out=n0[:], in0=pmm[0][:], in1=attT[0][0:Z, :], op=ALU.mult)
            n1 = smp.tile([Z, S], F32, tag="sm")
            nc.vector.tensor_tensor(out=n1[:], in0=pmm[1][:], in1=attT[1][0:Z, :], op=ALU.mult)
            nsum = smp.tile([Z, S], F32, tag="sm")
            nc.vector.tensor_tensor(out=nsum[:], in0=n0[:], in1=n1[:], op=ALU.add)
            jmu = smp.tile([Z, S], F32, tag="sm")
            nc.vector.tensor_tensor(out=jmu[:], in0=jv[:], in1=nsum[:], op=ALU.mult)

            predictor(jmu[:], 2, 0)

        # ---------- emission: software-pipelined over reps ----------------
        # Rep r's epilogue (E/predictors/G, which wait on the attention RS)
        # is emitted AFTER rep r+1's compute so its long-latency waits don't
        # block the next rep's work in the in-order engine queues.
        def epilogue(par):
            stage_E(0, par)
            predictor(attT[0][0:Z, :], 0, Y)
            stage_E(1, par)
            predictor(attT[1][0:Z, :], 1, 2 * Y)
            stage_G()

        # GCN1 for modality m, split into emission pieces so the NEXT rep's
        # GCN1 can interleave into the current rep's attention chunks (fills
        # PE's attention-phase idle slots and fires next-rep RS1 a phase
        # early). Identity reordering at K_REPS=1.
        def gcn1_pieces(m):
            pieces = [lambda m=m: stage_A(m)]
            for cp in range(NCH // 2):
                pieces.append(
                    lambda m=m, cp=cp: stage_agg_cp(m, s1sb[m][:], rs1_in[m],
                                                    cp, all_dve=True))
            pieces.append(
                lambda m=m: rs_collective(rs1_in[m], rs1_out[m]))
            return pieces

        REPS = int(os.environ.get("K_REPS", "1"))
        pending = None
        pre_done = False
        for _rep in range(REPS):
            par = _rep % 2
            if not pre_done:
                for f in gcn1_pieces(0):
                    f()
                for f in gcn1_pieces(1):
                    f()
            stage_B(0, rs1_out[0], b1[0], x1T[0][:])
            stage_C(0)
            stage_agg(0, s2sb[0][:], rs2_in[0])
            rs_collective(rs2_in[0], rs2_out[0])
            stage_B(1, rs1_out[1], b1[1], x1T[1][:])
            stage_C(1)
            stage_agg(1, s2sb[1][:], rs2_in[1])
            rs_collective(rs2_in[1], rs2_out[1])
            stage_B(0, rs2_out[0], b2[0], x2T[0][:])
            stage_D(0)
            ag_collective(0)
            stage_B(1, rs2_out[1], b2[1], x2T[1][:])
            stage_D(1)
            ag_collective(1)
            last = (_rep == REPS - 1)
            nxt0 = [] if last else gcn1_pieces(0)
            nxt1 = [] if last else gcn1_pieces(1)
            a1call0 = stage_F_pre(0)
            for ch in range(NCH):
                stage_F_ch(0, ch, a1call0, par)
                if ch < len(nxt0):
                    nxt0[ch]()
            a1call1 = stage_F_pre(1)
            for ch in range(NCH):
                stage_F_ch(1, ch, a1call1, par)
                if ch < len(nxt1):
                    nxt1[ch]()
            rs_collective(rsA_in[par], rsA_out[par])
            if pending is not None:
                epilogue(pending)
            pending = par
            pre_done = not last
        epilogue(pending)


def _build_nc():
    nc = bacc.Bacc("TRN2", target_bir_lowering=False, debug=False,
                   num_devices=NCORES)
    P = {}
    for m in range(M):
        P[f"adjTc{m}"] = nc.dram_tensor(f"adjTc{m}", [S, N], MM_DT, kind="ExternalInput").ap()
        P[f"xTc{m}"] = nc.dram_tensor(f"xTc{m}", [DP, S], MM_DT, kind="ExternalInput").ap()
        P[f"gc1_W{m}"] = nc.dram_tensor(f"gc1_W{m}", [DP, H], MM_DT, kind="ExternalInput").ap()
        P[f"gc1_b{m}"] = nc.dram_tensor(f"gc1_b{m}", [H, 1], F32, kind="ExternalInput").ap()
        P[f"gc2_W{m}"] = nc.dram_tensor(f"gc2_W{m}", [H, H], MM_DT, kind="ExternalInput").ap()
        P[f"gc2_b{m}"] = nc.dram_tensor(f"gc2_b{m}", [H, 1], F32, kind="ExternalInput").ap()
        P[f"gat_W{m}"] = nc.dram_tensor(f"gat_W{m}", [H, F2], MM_DT, kind="ExternalInput").ap()
        P[f"wa{m}"] = nc.dram_tensor(f"wa{m}", [H, 2], MM_DT, kind="ExternalInput").ap()
        P[f"spW1_{m}"] = nc.dram_tensor(f"spW1_{m}", [Z, PH], F32, kind="ExternalInput").ap()
        P[f"spb1_{m}"] = nc.dram_tensor(f"spb1_{m}", [PH, 1], F32, kind="ExternalInput").ap()
        P[f"spW2_{m}"] = nc.dram_tensor(f"spW2_{m}", [PH, Y], F32, kind="ExternalInput").ap()
        P[f"spb2_{m}"] = nc.dram_tensor(f"spb2_{m}", [Y, 1], F32, kind="ExternalInput").ap()
    P["jpW1"] = nc.dram_tensor("jpW1", [Z, PH], F32, kind="ExternalInput").ap()
    P["jpb1"] = nc.dram_tensor("jpb1", [PH, 1], F32, kind="ExternalInput").ap()
    P["jpW2"] = nc.dram_tensor("jpW2", [PH, Y], F32, kind="ExternalInput").ap()
    P["jpb2"] = nc.dram_tensor("jpb2", [Y, 1], F32, kind="ExternalInput").ap()
    P["maskT"] = nc.dram_tensor("maskT", [1, M * S], F32, kind="ExternalInput").ap()
    P["outT"] = nc.dram_tensor("outT", [3 * Y, S], F32, kind="ExternalOutput").ap()

    with tile.TileContext(nc) as tc:
        _emit(nc, tc, P)
    nc.compile()
    return nc


@functools.lru_cache(maxsize=1)
def _get_compiled():
    nc = _build_nc()
    nc.m = get_hw_module(nc.m)
    return nc


def _mm_np(a):
    if MM_BF16:
        import ml_dtypes
        return np.ascontiguousarray(a.astype(ml_dtypes.bfloat16))
    return np.ascontiguousarray(np.asarray(a, np.float32))


def _shard_inputs(inputs):
    f = np.float32
    in_maps = []
    pad_w = []
    for m in range(M):
        w = np.zeros((DP, H), f)
        w[:D, :] = inputs[f"gc1_W{m}"]
        pad_w.append(_mm_np(w))
    for c in range(NCORES):
        r0, r1 = c * S, (c + 1) * S
        im = {}
        for m in range(M):
            im[f"adjTc{m}"] = _mm_np(np.asarray(inputs[f"adj{m}"], f)[:, r0:r1].T)
            xp = np.zeros((DP, S), f)
            xp[:D, :] = np.asarray(inputs[f"x{m}"], f)[r0:r1, :].T
            im[f"xTc{m}"] = _mm_np(xp)
            im[f"gc1_W{m}"] = pad_w[m]
            im[f"gc1_b{m}"] = np.asarray(inputs[f"gc1_b{m}"], f).reshape(H, 1)
            im[f"gc2_W{m}"] = _mm_np(np.asarray(inputs[f"gc2_W{m}"], f))
            im[f"gc2_b{m}"] = np.asarray(inputs[f"gc2_b{m}"], f).reshape(H, 1)
            im[f"gat_W{m}"] = _mm_np(np.asarray(inputs[f"gat_W{m}"], f))
            ga_full = np.asarray(inputs[f"gat_a{m}"], f)
            wa = np.asarray(inputs[f"gat_W{m}"], f) @ np.stack(
                [ga_full[:F2, 0], ga_full[F2:, 0]], axis=1)
            im[f"wa{m}"] = _mm_np(wa)
            im[f"spW1_{m}"] = np.ascontiguousarray(np.asarray(inputs[f"spW1_{m}"], f))
            im[f"spb1_{m}"] = np.asarray(inputs[f"spb1_{m}"], f).reshape(PH, 1)
            im[f"spW2_{m}"] = np.ascontiguousarray(np.asarray(inputs[f"spW2_{m}"], f))
            im[f"spb2_{m}"] = np.asarray(inputs[f"spb2_{m}"], f).reshape(Y, 1)
        im["jpW1"] = np.ascontiguousarray(np.asarray(inputs["jpW1"], f))
        im["jpb1"] = np.asarray(inputs["jpb1"], f).reshape(PH, 1)
        im["jpW2"] = np.ascontiguousarray(np.asarray(inputs["jpW2"], f))
        im["jpb2"] = np.asarray(inputs["jpb2"], f).reshape(Y, 1)
        im["maskT"] = np.ascontiguousarray(
            np.asarray(inputs["mask"], f)[r0:r1, :].T.reshape(1, M * S))
        in_maps.append(im)
    return in_maps


def run(inputs, trace=False):
    nc = _get_compiled()
    in_maps = _shard_inputs(inputs)
    res = run_bass_kernel_spmd(nc, in_maps, list(range(NCORES)), trace=trace)
    out = np.zeros((N, 3 * Y), np.float32)
    for c in range(NCORES):
        out[c * S:(c + 1) * S, :] = res.results[c]["outT"].T
    return out, res


def kernel(**inputs):
    out, _ = run(inputs)
    return out
